# revision 30
# baseline (speedup 1.0000x reference)
"""DeepSeek hybrid sparse attention (CSA layer) Bass/Tile kernel for TRN2.

Sharding: 8 cores = batch (2) x sequence-chunk (4). Each core handles 512
tokens of one batch element: all projections, its slice of compressed K/V,
indexer keys; AllGather of compressed tensors within each 4-core batch
group; then dense-masked attention over the 512 compressed groups with
on-device top-64 selection; grouped output projection.

All activations on-chip are feature-major ([feature, token]) so matmuls
chain without transposes (weights stationary as lhsT).

Precision: the indexer chain (iq/ik/ig projections) runs as a 3-term fp16
hi/lo split:
    P1 = Wh.xh            (one PSUM bank)
    P2 = Wh.xl2k + Wl2k.xh  (second bank, lo parts pre-scaled by 2^11)
    W.x ~= P1 + 2^-11 P2
which carries ~22 mantissa bits (max iscore deviation vs fp32 < 1e-6,
verified to reproduce the fp32 top-64 selection exactly on this input) at
3 PE cycles/row instead of fp32's 4. Pooling, rms, iscore and top-k stay
fp32. The value chain (q/k/v, attention, output projection) runs in fp16
with fp32 accumulation; softmax denominators and rms scales in fp32.

DMA discipline: each HWDGE descriptor-generation costs ~625 ns on a shared
device, so weight strips are fused into [128, 1024] panels (ck|cv and
Wh|Wl pairs are interleaved host-side), x-hi loads as one DMA, small
constants are packed, and the post-AllGather retrievals ride the software
DGE (Pool) queue. Weight strips keep the SP queue to themselves; x,
constants, AllGather staging and output writes use the Activation queue.
"""

import numpy as np
import ml_dtypes
import concourse.bass as bass
import concourse.mybir as mybir
import concourse.tile as tile
from concourse import bacc

F32 = mybir.dt.float32
F16 = mybir.dt.float16
AF = mybir.ActivationFunctionType
ALU = mybir.AluOpType
F16NP = np.float16

# model dims
B, T, C = 2, 2048, 2048
NH, NKV, HD = 16, 8, 128
RATIO = 4
G = T // RATIO            # 512 compressed groups (full)
IDX_NH, IDX_HD = 16, 64
TOPK = 64
QR = 1024                 # q lowrank
ORPG = 1024               # o_proj rank
TC = 512                  # tokens per core
GC = 128                  # groups per core
NCORE = 8
NEGM = -30000.0           # additive causal mask value (exp -> 0 in fp32)
ZAP = -1.0e9              # top-k zap sentinel
SEL_THR = -5.0e8          # detection threshold for zapped entries
EPS = 1e-6
LSHIFT = float(2.0 ** 11)   # fp16 split lo-part scale

IDX_SCALE = float(np.float32(IDX_HD ** -0.5) / np.float32(IDX_NH))
ATT_SCALE = float(np.float32(HD ** -0.5))


def build_program(single_core=False):
    nc = bacc.Bacc("TRN2", target_bir_lowering=False, debug=False,
                   num_devices=1 if single_core else NCORE)
    dram = {}

    def din(name, shape, dtype=F32):
        dram[name] = nc.dram_tensor(name, shape, dtype, kind="ExternalInput").ap()
        return dram[name]

    din("xh", [C, TC], F16)              # fp16 hi part of x
    din("xl", [C, TC], F16)              # fp16 lo part (x - xh) * 2^11
    din("qa_w", [C, QR], F16)
    din("qb_w", [QR, NH * HD], F16)
    din("ckv_w", [C, 2 * NKV * HD], F16)   # [ck mg0|cv mg0|ck mg1|cv mg1]
    din("cg_w", [C, NKV * HD], F16)
    din("ikf_w", [C, 2048], F16)           # [Wh mg0|Wl mg0|Wh mg1|Wl mg1]
    din("igf_w", [C, 2048], F16)
    din("iqf_w", [C, 2048], F16)
    din("owaT", [C, ORPG], F16)
    din("opb", [ORPG, C], F16)
    din("csq1", [64, TC])                # rows: cos(32) then sin(32)
    din("csq2", [64, TC])                # rows: sin(32) then cos(32)
    din("csg1", [64, GC])
    din("csg2", [64, GC])
    din("apegf", [128, 32], F32)         # gate ape [d, kv*R]
    din("iapegf", [128, 32], F32)        # indexer gate ape [d, ft*R]
    din("causadd", [TC, G], F16)         # token-major additive (-30000/0)
    din("caus01T", [G, TC], F16)         # g-major multiplicative (1/0)
    din("eblk", [16, 1024])              # head-block indicator
    din("eblkT2", [128, 128])            # fused ebT blocks [128, 8*16]
    din("onesk", [128, 1])
    din("oneskh", [128, 1], F16)
    din("ident", [128, 128])
    din("sink", [1, 16])
    yT = nc.dram_tensor("yT", [C, TC], F32, kind="ExternalOutput").ap()

    with tile.TileContext(nc) as tc:
        _emit(nc, tc, dram, yT, single_core=single_core)
    nc.compile()
    return nc


def _emit(nc, tc, d, yT, single_core=False):
    import contextlib
    ctx = contextlib.ExitStack()
    with ctx:
        mem = ctx.enter_context(tc.tile_pool(name="mem", bufs=1))
        psum = ctx.enter_context(tc.tile_pool(name="ps", bufs=1, space="PSUM"))
        dpool = ctx.enter_context(tc.tile_pool(name="dram", bufs=1, space="DRAM"))

        def mt(shape, dtype, tag, name, bufs=None):
            return mem.tile(shape, dtype, tag=tag, name=name, bufs=bufs)

        def pt(tag, name, shape=(128, TC)):
            return psum.tile(list(shape), F32, tag=tag, name=name)

        def cload(name, shape, src, dtype=F32):
            t = mem.tile(shape, dtype, tag=name, name=name)
            nc.scalar.dma_start(t[:], src)
            return t

        # ---------- resident x (fp16 hi in one big tile) ----------
        xhb = mt([128, 16 * TC], F16, "xhb", "xhb")
        for i in range(16):
            nc.scalar.dma_start(xhb[:, i * TC:(i + 1) * TC],
                                d["xh"][i * 128:(i + 1) * 128, :])
        xh_s = [xhb[:, i * TC:(i + 1) * TC] for i in range(16)]
        apegf = cload("apegf_t", [128, 32], d["apegf"][:])
        csg1 = mt([128, GC], F32, "csg1_t", "csg1_t")
        nc.scalar.dma_start(csg1[64:128, :], d["csg1"][:])
        csg2 = mt([128, GC], F32, "csg2_t", "csg2_t")
        nc.scalar.dma_start(csg2[64:128, :], d["csg2"][:])
        ident = cload("ident_t", [128, 128], d["ident"][:])
        # x lo: 16 tiles on a ring later reused by ckrF / Mt
        xl_t = []
        for i in range(16):
            t = mt([128, TC], F16, "xl", f"xl{i}", bufs=16)
            nc.scalar.dma_start(t[:], d["xl"][i * 128:(i + 1) * 128, :])
            xl_t.append(t)
        xl_s = [t[:] for t in xl_t]
        iapegf = cload("iapegf_t", [128, 32], d["iapegf"][:])
        eblk = cload("eblk_t", [16, 1024], d["eblk"][:])
        ebT2 = cload("ebT2_t", [128, 128], d["eblkT2"][:])
        onesk = cload("onesk_t", [128, 1], d["onesk"][:])
        oneskh = cload("oneskh_t", [128, 1], d["oneskh"][:], F16)
        sinkt = cload("sink_t", [1, 16], d["sink"][:])
        expsink = mt([1, 16], F32, "expsink", "expsink")
        nc.scalar.activation(expsink[:], sinkt[:], AF.Exp)
        epsb = mt([128, 1], F32, "epsb", "epsb")
        nc.vector.memset(epsb[:], EPS)

        # ---------- projection passes ----------
        def pass8(pname, w, colh, K, rhs, consumer):
            """8 output tiles from w cols [colh*1024, (colh+1)*1024), one
            fused [128,1024] strip DMA per ki."""
            pss = [pt(f"b{j}", f"{pname}ps{colh}_{j}") for j in range(8)]
            nk = K // 128
            for ki in range(nk):
                ws = mt([128, 1024], F16, "wstrip", f"{pname}w{colh}_{ki}",
                        bufs=3)
                nc.sync.dma_start(
                    ws[:], w[ki * 128:(ki + 1) * 128,
                             colh * 1024:(colh + 1) * 1024])
                rt = rhs[ki]
                for j in range(8):
                    nc.tensor.matmul(pss[j][:], ws[:, j * 128:(j + 1) * 128],
                                     rt, start=(ki == 0), stop=(ki == nk - 1))
            for j in range(8):
                consumer(colh * 8 + j, pss[j])

        def pass_idx(pname, wf, mg, consumer):
            """fp16-split pass: 4 out tiles, strip = [Wh | Wl] for this mg.
            P1 (b0-3) = Wh.xh ; P2 (b4-7) = Wh.xl2k + Wl2k.xh."""
            pss = [pt(f"b{j}", f"{pname}ps{mg}_{j}") for j in range(4)]
            ps2 = [pt(f"b{4 + j}", f"{pname}pl{mg}_{j}") for j in range(4)]
            for ki in range(16):
                ws = mt([128, 1024], F16, "wstrip", f"{pname}w{mg}_{ki}",
                        bufs=3)
                nc.sync.dma_start(
                    ws[:], wf[ki * 128:(ki + 1) * 128,
                              mg * 1024:(mg + 1) * 1024])
                for j in range(4):
                    nc.tensor.matmul(pss[j][:], ws[:, j * 128:(j + 1) * 128],
                                     xh_s[ki], start=(ki == 0),
                                     stop=(ki == 15))
                for j in range(4):
                    nc.tensor.matmul(ps2[j][:], ws[:, j * 128:(j + 1) * 128],
                                     xl_s[ki], start=(ki == 0), stop=False)
                    nc.tensor.matmul(ps2[j][:],
                                     ws[:, 512 + j * 128:512 + (j + 1) * 128],
                                     xh_s[ki], start=False, stop=(ki == 15))
            for j in range(4):
                consumer(mg * 4 + j, pss[j], ps2[j])

        # ================= compressor (fp16 value path) =================
        ckr_p, cvg_p = [None] * 8, [None] * 8
        kvg = {}

        def make_ckv_consumer(mg):
            def cons(j8, ps):
                j = j8 - mg * 8
                if j < 4:
                    t = mt([128, TC], F16, "famb", f"ksb{mg * 4 + j}", bufs=16)
                    nc.scalar.copy(t[:], ps[:])
                    kvg[("k", mg * 4 + j)] = t
                else:
                    t = mt([128, TC], F16, "famb", f"vsb{mg * 4 + j - 4}",
                           bufs=16)
                    nc.scalar.copy(t[:], ps[:])
                    kvg[("v", mg * 4 + j - 4)] = t
            return cons

        def pool_head(kv):
            g_sb = kvg[("g", kv)]
            eg = mt([128, TC], F16, "eg", f"eg{kv}", bufs=2)
            nc.scalar.activation(eg[:], g_sb[:], AF.Exp)
            esum = mt([128, GC], F32, "esum", f"esum{kv}", bufs=2)
            nc.vector.tensor_reduce(esum[:],
                                    eg[:].rearrange("p (g r) -> p g r", r=RATIO),
                                    axis=mybir.AxisListType.X, op=ALU.add)
            erec = mt([128, GC], F32, "erec", f"erec{kv}", bufs=2)
            nc.vector.reciprocal(erec[:], esum[:])

            def pool_one(src, tag):
                kw = mt([128, TC], F16, "kw", f"kw_{tag}{kv}", bufs=1)
                nc.vector.tensor_mul(kw[:], src[:], eg[:])
                ks = mt([128, GC], F32, "ks", f"ks_{tag}{kv}", bufs=2)
                nc.vector.tensor_reduce(
                    ks[:], kw[:].rearrange("p (g r) -> p g r", r=RATIO),
                    axis=mybir.AxisListType.X, op=ALU.add)
                kp = mt([128, GC], F32, f"kp_{tag}", f"kp_{tag}{kv}", bufs=2)
                nc.vector.tensor_mul(kp[:], ks[:], erec[:])
                return kp

            ck_p = pool_one(kvg[("k", kv)], "k")
            cv_p = pool_one(kvg[("v", kv)], "v")

            # rope on pooled keys (rows 64:128); output fp32 for AllGather
            ckr = mt([128, GC], F32, "ckrp", f"ckr{kv}", bufs=8)
            nc.scalar.copy(ckr[0:64, :], ck_p[0:64, :])
            t1 = mt([32, GC], F32, "grt", f"rt1g{kv}", bufs=4)
            t2 = mt([32, GC], F32, "grt", f"rt2g{kv}", bufs=4)
            nc.vector.tensor_mul(t1[:], ck_p[64:96, :], csg1[64:96, :])
            nc.vector.tensor_mul(t2[:], ck_p[96:128, :], csg1[96:128, :])
            nc.vector.tensor_add(ckr[64:96, :], t1[:], t2[:])
            t3 = mt([32, GC], F32, "grt", f"rt3g{kv}", bufs=4)
            t4 = mt([32, GC], F32, "grt", f"rt4g{kv}", bufs=4)
            nc.vector.tensor_mul(t3[:], ck_p[64:96, :], csg2[64:96, :])
            nc.vector.tensor_mul(t4[:], ck_p[96:128, :], csg2[96:128, :])
            nc.vector.tensor_sub(ckr[96:128, :], t4[:], t3[:])
            ckr_p[kv] = ckr

            # transpose pooled values to g-major (fp32 for AllGather)
            pst = pt("b6", f"tps{kv}", (128, GC))
            nc.tensor.transpose(pst[:], cv_p[:], ident[:])
            cvg = mt([128, GC], F32, "cvgp", f"cvg{kv}", bufs=8)
            nc.vector.tensor_copy(cvg[:], pst[:])
            cvg_p[kv] = cvg

        def cg_cons(kv, ps):
            t = mt([128, TC], F16, "gt", f"gsb{kv}", bufs=4)
            ape = apegf[:, kv * 4:(kv + 1) * 4].unsqueeze(1).to_broadcast(
                [128, GC, RATIO])
            nc.vector.tensor_add(
                t[:].rearrange("p (g r) -> p g r", r=RATIO),
                ps[:].rearrange("p (g r) -> p g r", r=RATIO), ape)
            kvg[("g", kv)] = t
            pool_head(kv)

        pass8("ckv", d["ckv_w"], 0, C, xh_s, make_ckv_consumer(0))
        pass8("ckv", d["ckv_w"], 1, C, xh_s, make_ckv_consumer(1))
        pass8("cg", d["cg_w"], 0, C, xh_s, lambda j, ps: cg_cons(j, ps))

        # ================= indexer keys (fp16-split -> fp32) =============
        iksg = {}

        def ik_cons(key):
            def cons(mi, ps, ps2):
                t = mt([128, TC], F32, "famc", f"{key}sb{mi}", bufs=8)
                t2s = mt([128, TC], F32, "plo", f"{key}lo{mi}", bufs=2)
                nc.scalar.activation(t2s[:], ps2[:], AF.Copy,
                                     scale=1.0 / LSHIFT)
                nc.vector.tensor_add(t[:], t2s[:], ps[:])
                if key == "ig":
                    ape = iapegf[:, mi * 4:(mi + 1) * 4].unsqueeze(1) \
                        .to_broadcast([128, GC, RATIO])
                    tr = t[:].rearrange("p (g r) -> p g r", r=RATIO)
                    nc.vector.tensor_add(tr, tr, ape)
                iksg[(key, mi)] = t
            return cons

        ikp_t, iksq_t = [None] * 8, [None] * 8

        def ipool(ft):
            eg = mt([128, TC], F32, "ieg", f"ieg{ft}", bufs=1)
            nc.scalar.activation(eg[:], iksg[("ig", ft)][:], AF.Exp)
            esum = mt([128, GC], F32, "esum", f"iesum{ft}", bufs=2)
            nc.vector.tensor_reduce(esum[:],
                                    eg[:].rearrange("p (g r) -> p g r", r=RATIO),
                                    axis=mybir.AxisListType.X, op=ALU.add)
            erec = mt([128, GC], F32, "erec", f"ierec{ft}", bufs=2)
            nc.vector.reciprocal(erec[:], esum[:])
            kw = mt([128, TC], F32, "ikw", f"ikw{ft}", bufs=1)
            nc.vector.tensor_mul(kw[:], iksg[("ik", ft)][:], eg[:])
            ks = mt([128, GC], F32, "ks", f"iks{ft}", bufs=2)
            nc.vector.tensor_reduce(ks[:],
                                    kw[:].rearrange("p (g r) -> p g r", r=RATIO),
                                    axis=mybir.AxisListType.X, op=ALU.add)
            ikp = mt([128, GC], F32, "iknp", f"ikp{ft}", bufs=8)
            nc.vector.tensor_mul(ikp[:], ks[:], erec[:])
            ikp_t[ft] = ikp

        for mg in range(2):
            pass_idx("ik", d["ikf_w"], mg, ik_cons("ik"))
            pass_idx("ig", d["igf_w"], mg, ik_cons("ig"))
            for j in range(4):
                ipool(mg * 4 + j)

        # rms over each idx head (64 feats): ssq via block-diag ones matmul.
        # square and accumulate alternate so the 2-slot sqs ring never blocks
        # behind the accumulation matmuls.
        ps_ssq = pt("b4", "issq", (16, GC))
        for ft in range(8):
            iksq = mt([128, GC], F32, "sqs", f"iksq{ft}", bufs=2)
            nc.scalar.activation(iksq[:], ikp_t[ft][:], AF.Square)
            nc.tensor.matmul(ps_ssq[:], ebT2[:, ft * 16:(ft + 1) * 16],
                             iksq[:], start=(ft == 0), stop=(ft == 7))
        s_sqrt = mt([16, GC], F32, "s_ik_a", "s_ik_a")
        nc.scalar.activation(s_sqrt[:], ps_ssq[:], AF.Sqrt,
                             scale=1.0 / IDX_HD, bias=epsb[:16, :])
        s_ik = mt([16, GC], F32, "s_ik", "s_ik")
        nc.vector.reciprocal(s_ik[:], s_sqrt[:])
        for ft in range(8):
            psb = pt("b6", f"ibc{ft}", (128, GC))
            nc.tensor.matmul(psb[:], eblk[:, ft * 128:(ft + 1) * 128], s_ik[:],
                             start=True, stop=True)
            nc.vector.tensor_mul(ikp_t[ft][:], ikp_t[ft][:], psb[:])

        # ---------- AllGather of (ckr | ikn | cv_gmajor), all fp32 ----------
        agin = dpool.tile([3072, GC], F32, name="agin")
        for kv in range(8):
            nc.scalar.dma_start(agin[128 * kv:128 * (kv + 1), :], ckr_p[kv][:])
        for ft in range(8):
            nc.scalar.dma_start(agin[1024 + 128 * ft:1024 + 128 * (ft + 1), :],
                                ikp_t[ft][:])
        cvsec = agin[2048:3072, :].rearrange("(g kv) d -> g kv d", kv=8)
        for kv in range(8):
            nc.scalar.dma_start(cvsec[:, kv, :], cvg_p[kv][:])
        if not single_core:
            agout = dpool.tile([4 * 3072, GC], F32, name="agout")
            nc.gpsimd.collective_compute(
                "AllGather", ALU.bypass,
                replica_groups=[[0, 1, 2, 3], [4, 5, 6, 7]],
                ins=[agin.opt()], outs=[agout.opt()],
            )

        # ---------- retrieve gathered tensors (SWDGE / Pool queue) --------
        # ckrF / Mt reuse the xl ring (xl dies at the end of the iq pass).
        vvt = []
        for c in range(4):
            t = mt([128, 1024], F16, "vvt", f"vvt{c}", bufs=4)
            if single_core:
                nc.gpsimd.dma_start(
                    t[:], agin[2048:3072, :]
                    .rearrange("(g kv) d -> g (kv d)", kv=8))
            else:
                nc.gpsimd.dma_start(
                    t[:], agout[3072 * c + 2048:3072 * c + 3072, :]
                    .rearrange("(g kv) d -> g (kv d)", kv=8))
            vvt.append(t)
        iknF = []
        for ft in range(8):
            t = mt([128, G], F32, "iknf", f"iknF{ft}", bufs=8)
            tr = t[:].rearrange("p (c g) -> p c g", c=4)
            if single_core:
                for c in range(4):
                    nc.gpsimd.dma_start(
                        tr[:, c, :],
                        agin[1024 + 128 * ft:1024 + 128 * (ft + 1), :])
            else:
                nc.gpsimd.dma_start(
                    tr, agout[:].rearrange("(c s p) g -> s p c g",
                                           c=4, s=24, p=128)[8 + ft])
            iknF.append(t)
        # ================= q path (fp16, overlaps AG/retrieval) ==========
        csq1 = mt([128, TC], F32, "csq1_t", "csq1_t")
        nc.scalar.dma_start(csq1[64:128, :], d["csq1"][:])
        csq2 = mt([128, TC], F32, "csq2_t", "csq2_t")
        nc.scalar.dma_start(csq2[64:128, :], d["csq2"][:])
        qa_sb = [None] * 8

        def qa_cons(mi, ps):
            t = mt([128, TC], F16, "famb", f"qasb{mi}", bufs=16)
            nc.scalar.copy(t[:], ps[:])
            qa_sb[mi] = t

        pass8("qa", d["qa_w"], 0, C, xh_s, qa_cons)
        qa_s = [qa_sb[i][:] for i in range(8)]
        qr_t = [None] * 16

        def qb_cons(h, ps):
            qr = mt([128, TC], F16, f"qr{h}", f"qr{h}")
            nc.scalar.copy(qr[0:64, :], ps[0:64, :])
            t1 = mt([32, TC], F16, "qrt", f"qt1_{h}", bufs=4)
            t2 = mt([32, TC], F16, "qrt", f"qt2_{h}", bufs=4)
            nc.vector.tensor_mul(t1[:], ps[64:96, :], csq1[64:96, :])
            nc.vector.tensor_mul(t2[:], ps[96:128, :], csq1[96:128, :])
            nc.vector.tensor_add(qr[64:96, :], t1[:], t2[:])
            t3 = mt([32, TC], F16, "qrt", f"qt3_{h}", bufs=4)
            t4 = mt([32, TC], F16, "qrt", f"qt4_{h}", bufs=4)
            nc.vector.tensor_mul(t3[:], ps[64:96, :], csq2[64:96, :])
            nc.vector.tensor_mul(t4[:], ps[96:128, :], csq2[96:128, :])
            nc.vector.tensor_sub(qr[96:128, :], t4[:], t3[:])
            qr_t[h] = qr

        # ================= iq path (fp16-split -> fp32) =================
        iq_sb = [None] * 8

        def iq_cons(mi, ps, ps2):
            t = mt([128, TC], F32, "famc", f"iqsb{mi}", bufs=8)
            t2s = mt([128, TC], F32, "plo", f"iqlo{mi}", bufs=2)
            nc.scalar.activation(t2s[:], ps2[:], AF.Copy, scale=1.0 / LSHIFT)
            nc.vector.tensor_add(t[:], t2s[:], ps[:])
            iq_sb[mi] = t

        # interleave qb halves with iq groups: iq's PE work hides qb's
        # DVE-side rope drain.
        pass8("qb", d["qb_w"], 0, QR, qa_s, qb_cons)
        pass_idx("iq", d["iqf_w"], 0, iq_cons)
        pass8("qb", d["qb_w"], 1, QR, qa_s, qb_cons)
        pass_idx("iq", d["iqf_w"], 1, iq_cons)

        # ckrF retrieval reuses the xl ring, so it must be emitted after the
        # last xl reader (the iq pass above); overlaps the topk section.
        ckrF = []
        for kv in range(8):
            t = mt([128, G], F16, "xl", f"ckrF{kv}", bufs=16)
            tr = t[:].rearrange("p (c g) -> p c g", c=4)
            if single_core:
                for c in range(4):
                    nc.gpsimd.dma_start(
                        tr[:, c, :], agin[128 * kv:128 * (kv + 1), :])
            else:
                nc.gpsimd.dma_start(
                    tr, agout[:].rearrange("(c s p) g -> s p c g",
                                           c=4, s=24, p=128)[kv])
            ckrF.append(t)

        ps_qssq = pt("b4", "qssq", (16, TC))
        for ft in range(8):
            iqsq = mt([128, TC], F32, "sqs", f"iqsq{ft}", bufs=2)
            nc.scalar.activation(iqsq[:], iq_sb[ft][:], AF.Square)
            nc.tensor.matmul(ps_qssq[:], ebT2[:, ft * 16:(ft + 1) * 16],
                             iqsq[:], start=(ft == 0), stop=(ft == 7))
        sq_sqrt = mt([16, TC], F32, "s_iq_a", "s_iq_a")
        nc.scalar.activation(sq_sqrt[:], ps_qssq[:], AF.Sqrt,
                             scale=1.0 / IDX_HD, bias=epsb[:16, :])
        s_iq = mt([16, TC], F32, "s_iq", "s_iq")
        nc.vector.reciprocal(s_iq[:], sq_sqrt[:])
        for ft in range(8):
            psb = pt("b6", f"qbc{ft}")
            nc.tensor.matmul(psb[:], eblk[:, ft * 128:(ft + 1) * 128], s_iq[:],
                             start=True, stop=True)
            nc.vector.tensor_mul(iq_sb[ft][:], iq_sb[ft][:], psb[:])

        # ---------- indexer scores + top-64 selection (fp32) ----------
        caus01T = []
        for i in range(4):
            t = mt([128, TC], F16, f"caus01T{i}", f"caus01T{i}")
            nc.scalar.dma_start(t[:], d["caus01T"][i * 128:(i + 1) * 128, :])
            caus01T.append(t)
        Mt = [mt([128, TC], F16, "xl", f"msk{gt}", bufs=16) for gt in range(4)]
        for tt in range(4):
            causadd = mt([128, G], F16, "cauadd", f"causadd{tt}", bufs=2)
            nc.scalar.dma_start(causadd[:],
                                d["causadd"][tt * 128:(tt + 1) * 128, :])
            psi = pt("b4", f"iscp{tt}", (128, G))
            for ft in range(8):
                nc.tensor.matmul(psi[:],
                                 iq_sb[ft][:, tt * 128:(tt + 1) * 128],
                                 iknF[ft][:], start=(ft == 0), stop=(ft == 7))
            isc = mt([128, G], F32, "isc", f"isc{tt}", bufs=2)
            nc.vector.scalar_tensor_tensor(isc[:], psi[:], IDX_SCALE,
                                           causadd[:], op0=ALU.mult,
                                           op1=ALU.add)
            for r in range(8):
                mx = mt([128, 8], F32, "mx", f"mx{tt}_{r}", bufs=2)
                nc.vector.max(mx[:], isc[:])
                nc.vector.match_replace(isc[:], mx[:], isc[:], ZAP)
            nc.vector.tensor_scalar(isc[:], isc[:], SEL_THR, None,
                                    op0=ALU.is_le)
            for gt in range(4):
                pst = pt("b6", f"trp{gt}_{tt}", (128, 128))
                nc.tensor.transpose(pst[:],
                                    isc[:, gt * 128:(gt + 1) * 128], ident[:])
                nc.vector.tensor_mul(Mt[gt][:, tt * 128:(tt + 1) * 128], pst[:],
                                     caus01T[gt][:, tt * 128:(tt + 1) * 128])

        # ---------- attention per head (fp16 values, fp32 softmax den) -----
        outT = []
        for h in range(NH):
            kv = h // 2
            ps_den = pt("b4", f"aden{h}", (1, TC))
            ps_out = pt(f"b{5 + 2 * (h % 2)}", f"aout{h}")
            for gt in range(4):
                ps_s = pt(f"b{gt}", f"asc{h}_{gt}")
                nc.tensor.matmul(ps_s[:],
                                 ckrF[kv][:, gt * 128:(gt + 1) * 128],
                                 qr_t[h][:], start=True, stop=True)
                pu = mt([128, TC], F16, "pu", f"pu{h}_{gt}", bufs=4)
                nc.scalar.activation(pu[:], ps_s[:], AF.Exp, scale=ATT_SCALE)
                nc.vector.tensor_mul(pu[:], pu[:], Mt[gt][:])
                nc.tensor.matmul(ps_den[:], oneskh[:], pu[:], start=(gt == 0),
                                 stop=(gt == 3))
                nc.tensor.matmul(ps_out[:], vvt[gt][:, kv * 128:(kv + 1) * 128],
                                 pu[:], start=(gt == 0), stop=(gt == 3))
            den = mt([1, TC], F32, "den", f"den{h}", bufs=2)
            nc.vector.tensor_scalar(den[:], ps_den[:], expsink[0:1, h:h + 1],
                                    None, op0=ALU.add)
            rec = mt([1, TC], F32, "rec", f"rec{h}", bufs=2)
            nc.vector.reciprocal(rec[:], den[:])
            recB = mt([128, TC], F32, "recb", f"recb{h}", bufs=1)
            nc.gpsimd.partition_broadcast(recB[:], rec[:])
            # reuse head h's qr slot: qr[h] is dead after this head's score
            # matmuls, strictly before ot is written.
            ot = mt([128, TC], F16, f"qr{h}", f"outT{h}")
            nc.vector.tensor_mul(ot[:], ps_out[:], recB[:])
            outT.append(ot)
        outT_s = [t[:] for t in outT]

        # ---------- output projection ----------
        h_sb = [None] * 8

        def owa_cons(mi, ps):
            t = mt([128, TC], F16, "famb", f"hsb{mi}", bufs=16)
            nc.vector.tensor_copy(t[:], ps[:])
            h_sb[mi] = t

        pass8("owa", d["owaT"], 0, C, outT_s, owa_cons)
        h_s = [h_sb[i][:] for i in range(8)]
        ps_hssq = pt("b5", "hssq", (1, TC))
        for mi in range(8):
            hsq = mt([128, TC], F16, "sqs", f"hsq{mi}", bufs=2)
            nc.scalar.activation(hsq[:], h_sb[mi][:], AF.Square)
            nc.tensor.matmul(ps_hssq[:], oneskh[:], hsq[:],
                             start=(mi == 0), stop=(mi == 7))
        sh_sqrt = mt([1, TC], F32, "s_h_a", "s_h_a")
        nc.scalar.activation(sh_sqrt[:], ps_hssq[:], AF.Sqrt,
                             scale=1.0 / ORPG, bias=epsb[:1, :])
        s_h = mt([1, TC], F32, "s_h", "s_h")
        nc.vector.reciprocal(s_h[:], sh_sqrt[:])
        shB = mt([128, TC], F32, "shB", "shB")
        nc.gpsimd.partition_broadcast(shB[:], s_h[:])

        # y = (h @ opb) * rms_scale  (scale factored out of the contraction)
        def opb_cons(mi, ps):
            t = mt([128, TC], F32, "yo", f"yo{mi}", bufs=2)
            nc.vector.tensor_mul(t[:], ps[:], shB[:])
            nc.scalar.dma_start(yT[mi * 128:(mi + 1) * 128, :], t[:])

        pass8("opb", d["opb"], 0, ORPG, h_s, opb_cons)
        pass8("opb", d["opb"], 1, ORPG, h_s, opb_cons)


# ------------------------------------------------------------------
# host side
# ------------------------------------------------------------------

def make_host_constants():
    ge = np.arange(RATIO - 1, T, RATIO)             # group ends [G]
    pos = np.arange(T, dtype=np.float32)
    inv = 10000.0 ** (-np.arange(0, HD // 2, dtype=np.float32) / (HD // 2))
    ang = pos[:, None] * inv[None, :]               # [T, 64]
    cos_full = np.cos(ang).astype(np.float32)
    sin_full = np.sin(ang).astype(np.float32)
    consts = {}
    consts["eblk"] = np.zeros((16, 1024), np.float32)
    for hh in range(16):
        consts["eblk"][hh, hh * 64:(hh + 1) * 64] = 1.0
    consts["eblkT2"] = np.ascontiguousarray(
        consts["eblk"].T.reshape(8, 128, 16).transpose(1, 0, 2)
        .reshape(128, 128))
    consts["onesk"] = np.ones((128, 1), np.float32)
    consts["oneskh"] = np.ones((128, 1), F16NP)
    consts["ident"] = np.eye(128, dtype=np.float32)
    percore = []
    tarr = np.arange(T)
    for c in range(NCORE):
        q = c % 4
        t0 = TC * q
        g0 = GC * q
        pc = {}
        cq = cos_full[t0:t0 + TC, :32].T
        sq = sin_full[t0:t0 + TC, :32].T
        cg = cos_full[ge[g0:g0 + GC], :32].T
        sg = sin_full[ge[g0:g0 + GC], :32].T
        pc["csq1"] = np.ascontiguousarray(np.concatenate([cq, sq], 0))
        pc["csq2"] = np.ascontiguousarray(np.concatenate([sq, cq], 0))
        pc["csg1"] = np.ascontiguousarray(np.concatenate([cg, sg], 0))
        pc["csg2"] = np.ascontiguousarray(np.concatenate([sg, cg], 0))
        causal = (ge[None, :] <= tarr[t0:t0 + TC, None])   # [TC, G]
        pc["causadd"] = np.where(causal, 0.0, NEGM).astype(F16NP)
        pc["caus01T"] = np.ascontiguousarray(causal.T).astype(F16NP)
        percore.append(pc)
    return consts, percore


def _splitfuse16(w):
    """[C, 1024] fp32 -> [C, 2048] fp16 [Wh mg0|Wl mg0|Wh mg1|Wl mg1]."""
    w = np.asarray(w, np.float32)
    hi = w.astype(F16NP)
    lo = ((w - hi.astype(np.float32)) * np.float32(LSHIFT)).astype(F16NP)
    return np.ascontiguousarray(np.concatenate(
        [hi[:, :512], lo[:, :512], hi[:, 512:], lo[:, 512:]], axis=1))


_CACHED = {}


def get_program():
    if "nc" not in _CACHED:
        _CACHED["nc"] = build_program()
    return _CACHED["nc"]


def get_runner():
    """Cached jitted SPMD executable (mirrors bass2jax.run_bass_via_pjrt but
    builds the jax.jit once, so repeat calls skip retrace/relower)."""
    if "runner" in _CACHED:
        return _CACHED["runner"]
    import jax
    from jax.experimental.shard_map import shard_map
    from jax.sharding import Mesh, PartitionSpec
    import concourse.mybir as _mb
    from concourse.bass2jax import (_bass_exec_p, install_neuronx_cc_hook,
                                    partition_id_tensor)
    nc = get_program()
    install_neuronx_cc_hook()
    partition_name = (nc.partition_id_tensor.name
                      if nc.partition_id_tensor else None)
    in_names, out_names, out_avals, zero_shapes = [], [], [], []
    for alloc in nc.m.functions[0].allocations:
        if not isinstance(alloc, _mb.MemoryLocationSet):
            continue
        name = alloc.memorylocations[0].name
        if alloc.kind == "ExternalInput":
            if name != partition_name:
                in_names.append(name)
        elif alloc.kind == "ExternalOutput":
            shape = tuple(alloc.tensor_shape)
            dtype = _mb.dt.np(alloc.dtype)
            out_names.append(name)
            out_avals.append(jax.core.ShapedArray(shape, dtype))
            zero_shapes.append((shape, dtype))
    n_params = len(in_names)
    n_outs = len(out_avals)
    all_names = list(in_names) + list(out_names)
    if partition_name is not None:
        all_names.append(partition_name)
    donate = tuple(range(n_params, n_params + n_outs))

    def _body(*args):
        operands = list(args)
        if partition_name is not None:
            operands.append(partition_id_tensor())
        return tuple(_bass_exec_p.bind(
            *operands, out_avals=tuple(out_avals), in_names=tuple(all_names),
            out_names=tuple(out_names), lowering_input_output_aliases=(),
            sim_require_finite=True, sim_require_nnan=True, nc=nc))

    devices = jax.devices()[:NCORE]
    mesh = Mesh(np.asarray(devices), ("core",))
    in_specs = (PartitionSpec("core"),) * (n_params + n_outs)
    out_specs = (PartitionSpec("core"),) * n_outs
    sharded = jax.jit(
        shard_map(_body, mesh=mesh, in_specs=in_specs, out_specs=out_specs,
                  check_rep=False),
        donate_argnums=donate, keep_unused=True)

    def run(in_maps):
        concat_in = [
            np.concatenate([np.asarray(in_maps[c][nm]) for c in range(NCORE)],
                           axis=0)
            for nm in in_names]
        zeros = [np.zeros((NCORE * s[0], *s[1:]), dt)
                 for (s, dt) in zero_shapes]
        outs = sharded(*concat_in, *zeros)
        return [
            {nm: np.asarray(outs[i]).reshape(NCORE, *zero_shapes[i][0])[c]
             for i, nm in enumerate(out_names)}
            for c in range(NCORE)]

    _CACHED["runner"] = run
    return run


def kernel(x, cos, sin, q_a_w, q_b_w, ck_w, cv_w, cg_w, c_ape,
           iq_w, ik_w, ig_w, i_ape, sink, o_wa, o_pb):
    nc = get_program()
    x = np.asarray(x, np.float32)
    if "consts" not in _CACHED:
        _CACHED["consts"] = make_host_constants()
    consts, percore = _CACHED["consts"]
    c_ape = np.asarray(c_ape, np.float32)
    i_ape = np.asarray(i_ape, np.float32)
    # apegf: [128 (d within kv-head), 8 kv * RATIO]
    apegf = np.ascontiguousarray(
        c_ape.transpose(1, 2, 0).reshape(NKV, HD, RATIO)
        .transpose(1, 0, 2).reshape(HD, NKV * RATIO)).astype(np.float32)
    iape_t = i_ape.transpose(1, 2, 0).reshape(IDX_NH * IDX_HD, RATIO) \
        .reshape(8, 128, RATIO)
    iapegf = np.ascontiguousarray(
        iape_t.transpose(1, 0, 2).reshape(128, 8 * RATIO)).astype(np.float32)
    ck16 = np.asarray(ck_w, np.float32).astype(F16NP)
    cv16 = np.asarray(cv_w, np.float32).astype(F16NP)
    ckv = np.ascontiguousarray(np.concatenate(
        [ck16[:, :512], cv16[:, :512], ck16[:, 512:], cv16[:, 512:]], axis=1))
    shared = dict(
        qa_w=np.asarray(q_a_w, np.float32).astype(F16NP),
        qb_w=np.asarray(q_b_w, np.float32).astype(F16NP),
        ckv_w=ckv,
        cg_w=np.asarray(cg_w, np.float32).astype(F16NP),
        ikf_w=_splitfuse16(ik_w),
        igf_w=_splitfuse16(ig_w),
        iqf_w=_splitfuse16(iq_w),
        owaT=np.ascontiguousarray(
            np.asarray(o_wa, np.float32)[0].T).astype(F16NP),
        opb=np.asarray(o_pb, np.float32).astype(F16NP),
        apegf=apegf, iapegf=iapegf,
        sink=np.asarray(sink, np.float32).reshape(1, 16),
        **consts,
    )
    in_maps = []
    for c in range(NCORE):
        b, q = c // 4, c % 4
        m = dict(shared)
        xTc = np.ascontiguousarray(x[b, TC * q:TC * (q + 1), :].T)
        xh = xTc.astype(F16NP)
        m["xh"] = xh
        m["xl"] = ((xTc - xh.astype(np.float32)) *
                   np.float32(LSHIFT)).astype(F16NP)
        m.update(percore[c])
        in_maps.append(m)
    results = get_runner()(in_maps)
    y = np.empty((B, T, C), np.float32)
    for c in range(NCORE):
        b, q = c // 4, c % 4
        y[b, TC * q:TC * (q + 1), :] = results[c]["yT"].T
    return y


# revision 57
# speedup vs baseline: 1.1640x; 1.1640x over previous
"""DeepSeek hybrid sparse attention (CSA layer) Bass/Tile kernel for TRN2.

Sharding: 8 cores = batch (2) x sequence-chunk (4). Each core handles 512
tokens of one batch element: all projections, its slice of compressed K/V,
indexer keys; AllGather of compressed tensors within each 4-core batch
group; then dense-masked attention over the 512 compressed groups with
on-device top-64 selection; grouped output projection.

All activations on-chip are feature-major ([feature, token]) so matmuls
chain without transposes (weights stationary as lhsT).

Precision: the indexer chain (iq/ik/ig projections) runs as a 3-term fp16
hi/lo split:
    P1 = Wh.xh            (one PSUM bank)
    P2 = Wh.xl2k + Wl2k.xh  (second bank, lo parts pre-scaled by 2^11)
    W.x ~= P1 + 2^-11 P2
which carries ~22 mantissa bits (max iscore deviation vs fp32 < 1e-6,
verified to reproduce the fp32 top-64 selection exactly on this input) at
3 PE cycles/row instead of fp32's 4. Pooling, rms, iscore and top-k stay
fp32. The value chain (q/k/v, attention, output projection) runs in fp16
with fp32 accumulation; softmax denominators and rms scales in fp32.

DMA discipline: each HWDGE descriptor-generation costs ~625 ns on a shared
device, so weight strips are fused into [128, 1024] panels (ck|cv and
Wh|Wl pairs are interleaved host-side), x-hi loads as one DMA, small
constants are packed, and the post-AllGather retrievals ride the software
DGE (Pool) queue. Weight strips keep the SP queue to themselves; x,
constants, AllGather staging and output writes use the Activation queue.
"""

import numpy as np
import ml_dtypes
import concourse.bass as bass
import concourse.mybir as mybir
import concourse.tile as tile
from concourse import bacc

F32 = mybir.dt.float32
F16 = mybir.dt.float16
AF = mybir.ActivationFunctionType
ALU = mybir.AluOpType
F16NP = np.float16

# model dims
B, T, C = 2, 2048, 2048
NH, NKV, HD = 16, 8, 128
RATIO = 4
G = T // RATIO            # 512 compressed groups (full)
IDX_NH, IDX_HD = 16, 64
TOPK = 64
QR = 1024                 # q lowrank
ORPG = 1024               # o_proj rank
TC = 512                  # tokens per core
GC = 128                  # groups per core
NCORE = 8
NEGM = -30000.0           # additive causal mask value (exp -> 0 in fp32)
ZAP = -1.0e9              # top-k zap sentinel
SEL_THR = -5.0e8          # detection threshold for zapped entries
EPS = 1e-6
LSHIFT = float(2.0 ** 11)   # fp16 split lo-part scale

IDX_SCALE = float(np.float32(IDX_HD ** -0.5) / np.float32(IDX_NH))
ATT_SCALE = float(np.float32(HD ** -0.5))


def build_program(single_core=False):
    nc = bacc.Bacc("TRN2", target_bir_lowering=False, debug=False,
                   num_devices=1 if single_core else NCORE)
    dram = {}

    def din(name, shape, dtype=F32):
        dram[name] = nc.dram_tensor(name, shape, dtype, kind="ExternalInput").ap()
        return dram[name]

    din("xh", [C, TC], F16)              # fp16 hi part of x
    din("xl", [C, TC], F16)              # fp16 lo part (x - xh) * 2^11
    din("wq", [C, NH * HD], F16)         # host-fused q_a_w @ q_b_w
    din("ckv_w", [C, 2 * NKV * HD], F16)   # [ck mg0|cv mg0|ck mg1|cv mg1]
    din("cg_w", [C, NKV * HD], F16)
    din("ikf_w", [C, 2048], F16)           # [Wh mg0|Wl mg0|Wh mg1|Wl mg1]
    din("igf_w", [C, 2048], F16)
    din("iqf_w", [C, 2048], F16)
    din("owaT", [C, ORPG], F16)
    din("opb", [ORPG, C], F16)
    din("csqb", [64, 2 * TC], F16)       # [cos|sin ; sin|cos] rope table
    din("csg1", [64, GC], F16)
    din("csg2", [64, GC], F16)
    din("apegf", [128, 32], F32)         # gate ape [d, kv*R]
    din("iapegf", [128, 32], F32)        # indexer gate ape [d, ft*R]
    din("causadd", [TC, G], F16)         # token-major additive (-30000/0)
    din("caus01T", [G, TC], F16)         # g-major multiplicative (1/0)
    din("eblk", [16, 1024])              # head-block indicator
    din("eblkT2", [128, 128])            # fused ebT blocks [128, 8*16]
    din("onesk", [128, 1])
    din("oneskh", [128, 1], F16)
    din("ident", [128, 128])
    din("identh", [128, 128], F16)
    din("sink", [1, 16])
    yT = nc.dram_tensor("yT", [C, TC], F32, kind="ExternalOutput").ap()

    with tile.TileContext(nc) as tc:
        _emit(nc, tc, dram, yT, single_core=single_core)
    nc.compile()
    return nc


def _emit(nc, tc, d, yT, single_core=False):
    import contextlib
    ctx = contextlib.ExitStack()
    with ctx:
        mem = ctx.enter_context(tc.tile_pool(name="mem", bufs=1))
        psum = ctx.enter_context(tc.tile_pool(name="ps", bufs=1, space="PSUM"))
        dpool = ctx.enter_context(tc.tile_pool(name="dram", bufs=1, space="DRAM"))

        def mt(shape, dtype, tag, name, bufs=None):
            return mem.tile(shape, dtype, tag=tag, name=name, bufs=bufs)

        def pt(tag, name, shape=(128, TC), dtype=F32):
            return psum.tile(list(shape), dtype, tag=tag, name=name)

        def cload(name, shape, src, dtype=F32, eng=None):
            t = mem.tile(shape, dtype, tag=name, name=name)
            (eng or nc.scalar).dma_start(t[:], src)
            return t

        # ---------- resident x (fp16 hi in one big tile) ----------
        xhb = mt([128, 16 * TC], F16, "xhb", "xhb")
        for i in range(16):
            nc.scalar.dma_start(xhb[:, i * TC:(i + 1) * TC],
                                d["xh"][i * 128:(i + 1) * 128, :])
        xh_s = [xhb[:, i * TC:(i + 1) * TC] for i in range(16)]
        apegf = cload("apegf_t", [128, 32], d["apegf"][:])
        csg1 = mt([128, GC], F16, "csg1_t", "csg1_t")
        nc.scalar.dma_start(csg1[64:128, :], d["csg1"][:])
        csg2 = mt([128, GC], F16, "csg2_t", "csg2_t")
        nc.scalar.dma_start(csg2[64:128, :], d["csg2"][:])
        ident = cload("ident_t", [128, 128], d["ident"][:])
        identh = cload("identh_t", [128, 128], d["identh"][:], F16)
        # x lo: 16 tiles on a ring later reused by ckrF / Mt. Needed only
        # from the ik pass on, so it rides the Pool SWDGE queue and keeps
        # the Act HWDGE queue free for the compressor's PSUM drains.
        xl_t = []
        for i in range(16):
            t = mt([128, TC], F16, "xl", f"xl{i}", bufs=16)
            nc.gpsimd.dma_start(t[:], d["xl"][i * 128:(i + 1) * 128, :])
            xl_t.append(t)
        xl_s = [t[:] for t in xl_t]
        iapegf = cload("iapegf_t", [128, 32], d["iapegf"][:], eng=nc.gpsimd)
        eblk = cload("eblk_t", [16, 1024], d["eblk"][:], eng=nc.gpsimd)
        ebT2 = cload("ebT2_t", [128, 128], d["eblkT2"][:], eng=nc.gpsimd)
        onesk = cload("onesk_t", [128, 1], d["onesk"][:], eng=nc.gpsimd)
        oneskh = cload("oneskh_t", [128, 1], d["oneskh"][:], F16,
                       eng=nc.gpsimd)
        sinkt = cload("sink_t", [1, 16], d["sink"][:], eng=nc.gpsimd)
        expsink = mt([1, 16], F32, "expsink", "expsink")
        nc.scalar.activation(expsink[:], sinkt[:], AF.Exp)
        epsb = mt([128, 1], F32, "epsb", "epsb")
        nc.vector.memset(epsb[:], EPS)

        # ---------- projection passes ----------
        def pass8(pname, w, colh, K, rhs, consumer):
            """8 output tiles from w cols [colh*1024, (colh+1)*1024), one
            fused [128,1024] strip DMA per ki."""
            pss = [pt(f"b{j}", f"{pname}ps{colh}_{j}") for j in range(8)]
            nk = K // 128
            for ki in range(nk):
                ws = mt([128, 1024], F16, "wstrip", f"{pname}w{colh}_{ki}",
                        bufs=3)
                nc.sync.dma_start(
                    ws[:], w[ki * 128:(ki + 1) * 128,
                             colh * 1024:(colh + 1) * 1024])
                rt = rhs[ki]
                for j in range(8):
                    nc.tensor.matmul(pss[j][:], ws[:, j * 128:(j + 1) * 128],
                                     rt, start=(ki == 0), stop=(ki == nk - 1))
            if consumer is None:
                return pss
            for j in range(8):
                consumer(colh * 8 + j, pss[j])
            return pss

        def pass_idx(pname, wf, mg, consumer):
            """fp16-split pass: 4 out tiles, strip = [Wh | Wl] for this mg.
            P1 (b0-3) = Wh.xh ; P2 (b4-7) = Wh.xl2k + Wl2k.xh."""
            pss = [pt(f"b{j}", f"{pname}ps{mg}_{j}") for j in range(4)]
            ps2 = [pt(f"b{4 + j}", f"{pname}pl{mg}_{j}") for j in range(4)]
            for ki in range(16):
                ws = mt([128, 1024], F16, "wstrip", f"{pname}w{mg}_{ki}",
                        bufs=3)
                nc.sync.dma_start(
                    ws[:], wf[ki * 128:(ki + 1) * 128,
                              mg * 1024:(mg + 1) * 1024])
                for j in range(4):
                    nc.tensor.matmul(pss[j][:], ws[:, j * 128:(j + 1) * 128],
                                     xh_s[ki], start=(ki == 0),
                                     stop=(ki == 15))
                for j in range(4):
                    nc.tensor.matmul(ps2[j][:], ws[:, j * 128:(j + 1) * 128],
                                     xl_s[ki], start=(ki == 0), stop=False)
                    nc.tensor.matmul(ps2[j][:],
                                     ws[:, 512 + j * 128:512 + (j + 1) * 128],
                                     xh_s[ki], start=False, stop=(ki == 15))
            for j in range(4):
                consumer(mg * 4 + j, pss[j], ps2[j])

        # ================= compressor (fp16 value path) =================
        ckr_p, cvg_p = [None] * 8, [None] * 8
        kvg = {}

        def make_ckv_consumer(mg):
            def cons(j8, ps):
                j = j8 - mg * 8
                if j < 4:
                    t = mt([128, TC], F16, "famb", f"ksb{mg * 4 + j}", bufs=16)
                    nc.scalar.copy(t[:], ps[:])
                    kvg[("k", mg * 4 + j)] = t
                else:
                    t = mt([128, TC], F16, "famb", f"vsb{mg * 4 + j - 4}",
                           bufs=16)
                    nc.scalar.copy(t[:], ps[:])
                    kvg[("v", mg * 4 + j - 4)] = t
            return cons

        def pool_head(kv):
            g_sb = kvg[("g", kv)]
            eg = mt([128, TC], F16, "eg", f"eg{kv}", bufs=2)
            nc.scalar.activation(eg[:], g_sb[:], AF.Exp)
            # fp16 intermediates keep the DVE 2x 16-bit path (value chain)
            esum = mt([128, GC], F16, "esum", f"esum{kv}", bufs=2)
            with nc.allow_low_precision(reason="value-path 4-elem pool"):
                nc.vector.tensor_reduce(
                    esum[:], eg[:].rearrange("p (g r) -> p g r", r=RATIO),
                    axis=mybir.AxisListType.X, op=ALU.add)
            erec = mt([128, GC], F16, "erec", f"erec{kv}", bufs=2)
            with nc.allow_low_precision(reason="value-path pool softmax"):
                nc.vector.reciprocal(erec[:], esum[:])

            def pool_one(src, tag):
                kw = mt([128, TC], F16, "kw", f"kw_{tag}{kv}", bufs=1)
                nc.vector.tensor_mul(kw[:], src[:], eg[:])
                ks = mt([128, GC], F16, "ks", f"ks_{tag}{kv}", bufs=2)
                with nc.allow_low_precision(reason="value-path 4-elem pool"):
                    nc.vector.tensor_reduce(
                        ks[:], kw[:].rearrange("p (g r) -> p g r", r=RATIO),
                        axis=mybir.AxisListType.X, op=ALU.add)
                kp = mt([128, GC], F16, f"kp_{tag}", f"kp_{tag}{kv}", bufs=2)
                nc.vector.tensor_mul(kp[:], ks[:], erec[:])
                return kp

            ck_p = pool_one(kvg[("k", kv)], "k")
            cv_p = pool_one(kvg[("v", kv)], "v")

            # rope on pooled keys (rows 64:128); output fp32 for AllGather
            ckr = mt([128, GC], F32, "ckrp", f"ckr{kv}", bufs=8)
            nc.scalar.copy(ckr[0:64, :], ck_p[0:64, :])
            t1 = mt([32, GC], F16, "grt", f"rt1g{kv}", bufs=4)
            t2 = mt([32, GC], F16, "grt", f"rt2g{kv}", bufs=4)
            nc.vector.tensor_mul(t1[:], ck_p[64:96, :], csg1[64:96, :])
            nc.vector.tensor_mul(t2[:], ck_p[96:128, :], csg1[96:128, :])
            nc.vector.tensor_add(ckr[64:96, :], t1[:], t2[:])
            t3 = mt([32, GC], F16, "grt", f"rt3g{kv}", bufs=4)
            t4 = mt([32, GC], F16, "grt", f"rt4g{kv}", bufs=4)
            nc.vector.tensor_mul(t3[:], ck_p[64:96, :], csg2[64:96, :])
            nc.vector.tensor_mul(t4[:], ck_p[96:128, :], csg2[96:128, :])
            nc.vector.tensor_sub(ckr[96:128, :], t4[:], t3[:])
            ckr_p[kv] = ckr

            # transpose pooled values to g-major (fp32 for AllGather)
            pst = pt("b6", f"tps{kv}", (128, GC), dtype=F16)
            nc.tensor.transpose(pst[:], cv_p[:], identh[:])
            cvg = mt([128, GC], F32, "cvgp", f"cvg{kv}", bufs=8)
            nc.vector.tensor_copy(cvg[:], pst[:])
            cvg_p[kv] = cvg

        def cg_cons(kv, ps):
            t = mt([128, TC], F16, "gt", f"gsb{kv}", bufs=4)
            ape = apegf[:, kv * 4:(kv + 1) * 4].unsqueeze(1).to_broadcast(
                [128, GC, RATIO])
            nc.vector.tensor_add(
                t[:].rearrange("p (g r) -> p g r", r=RATIO),
                ps[:].rearrange("p (g r) -> p g r", r=RATIO), ape)
            kvg[("g", kv)] = t
            pool_head(kv)

        pass8("ckv", d["ckv_w"], 0, C, xh_s, make_ckv_consumer(0))
        pass8("ckv", d["ckv_w"], 1, C, xh_s, make_ckv_consumer(1))
        pass8("cg", d["cg_w"], 0, C, xh_s, lambda j, ps: cg_cons(j, ps))

        # ================= indexer keys (fp16-split -> fp32) =============
        iksg = {}

        def ik_cons(key):
            def cons(mi, ps, ps2):
                t = mt([128, TC], F32, "famc", f"{key}sb{mi}", bufs=8)
                t2s = mt([128, TC], F32, "plo", f"{key}lo{mi}", bufs=2)
                nc.scalar.activation(t2s[:], ps2[:], AF.Copy,
                                     scale=1.0 / LSHIFT)
                nc.vector.tensor_add(t[:], t2s[:], ps[:])
                if key == "ig":
                    ape = iapegf[:, mi * 4:(mi + 1) * 4].unsqueeze(1) \
                        .to_broadcast([128, GC, RATIO])
                    tr = t[:].rearrange("p (g r) -> p g r", r=RATIO)
                    nc.vector.tensor_add(tr, tr, ape)
                iksg[(key, mi)] = t
            return cons

        ikp_t, iksq_t = [None] * 8, [None] * 8

        def ipool(ft):
            eg = mt([128, TC], F32, "ieg", f"ieg{ft}", bufs=1)
            nc.scalar.activation(eg[:], iksg[("ig", ft)][:], AF.Exp)
            esum = mt([128, GC], F32, "esum", f"iesum{ft}", bufs=2)
            nc.vector.tensor_reduce(esum[:],
                                    eg[:].rearrange("p (g r) -> p g r", r=RATIO),
                                    axis=mybir.AxisListType.X, op=ALU.add)
            erec = mt([128, GC], F32, "erec", f"ierec{ft}", bufs=2)
            nc.vector.reciprocal(erec[:], esum[:])
            kw = mt([128, TC], F32, "ikw", f"ikw{ft}", bufs=1)
            nc.vector.tensor_mul(kw[:], iksg[("ik", ft)][:], eg[:])
            ks = mt([128, GC], F32, "ks", f"iks{ft}", bufs=2)
            nc.vector.tensor_reduce(ks[:],
                                    kw[:].rearrange("p (g r) -> p g r", r=RATIO),
                                    axis=mybir.AxisListType.X, op=ALU.add)
            ikp = mt([128, GC], F32, "iknp", f"ikp{ft}", bufs=8)
            nc.vector.tensor_mul(ikp[:], ks[:], erec[:])
            ikp_t[ft] = ikp

        for mg in range(2):
            pass_idx("ik", d["ikf_w"], mg, ik_cons("ik"))
            pass_idx("ig", d["igf_w"], mg, ik_cons("ig"))
            for j in range(4):
                ipool(mg * 4 + j)

        # rms over each idx head (64 feats): ssq via block-diag ones matmul.
        # square and accumulate alternate so the 2-slot sqs ring never blocks
        # behind the accumulation matmuls.
        ps_ssq = pt("b4", "issq", (16, GC))
        for ft in range(8):
            iksq = mt([128, GC], F32, "sqs", f"iksq{ft}", bufs=2)
            nc.scalar.activation(iksq[:], ikp_t[ft][:], AF.Square)
            nc.tensor.matmul(ps_ssq[:], ebT2[:, ft * 16:(ft + 1) * 16],
                             iksq[:], start=(ft == 0), stop=(ft == 7))
        s_sqrt = mt([16, GC], F32, "s_ik_a", "s_ik_a")
        nc.scalar.activation(s_sqrt[:], ps_ssq[:], AF.Sqrt,
                             scale=1.0 / IDX_HD, bias=epsb[:16, :])
        s_ik = mt([16, GC], F32, "s_ik", "s_ik")
        nc.vector.reciprocal(s_ik[:], s_sqrt[:])
        for ft in range(8):
            psb = pt("b6", f"ibc{ft}", (128, GC))
            nc.tensor.matmul(psb[:], eblk[:, ft * 128:(ft + 1) * 128], s_ik[:],
                             start=True, stop=True)
            nc.vector.tensor_mul(ikp_t[ft][:], ikp_t[ft][:], psb[:])

        # ---------- AllGather of (ckr | ikn | cv_gmajor), all fp32 ----------
        agin = dpool.tile([3072, GC], F32, name="agin")
        for kv in range(8):
            nc.gpsimd.dma_start(agin[128 * kv:128 * (kv + 1), :], ckr_p[kv][:])
        for ft in range(8):
            nc.gpsimd.dma_start(agin[1024 + 128 * ft:1024 + 128 * (ft + 1), :],
                                ikp_t[ft][:])
        cvsec = agin[2048:3072, :].rearrange("(g kv) d -> g kv d", kv=8)
        for kv in range(8):
            nc.gpsimd.dma_start(cvsec[:, kv, :], cvg_p[kv][:])
        if not single_core:
            agout = dpool.tile([4 * 3072, GC], F32, name="agout")
            nc.gpsimd.collective_compute(
                "AllGather", ALU.bypass,
                replica_groups=[[0, 1, 2, 3], [4, 5, 6, 7]],
                ins=[agin.opt()], outs=[agout.opt()],
            )

        # ---------- retrieve gathered tensors (SWDGE / Pool queue) --------
        # ckrF / Mt reuse the xl ring (xl dies at the end of the iq pass).
        vvt = []
        for c in range(4):
            t = mt([128, 1024], F16, "vvt", f"vvt{c}", bufs=4)
            if single_core:
                nc.gpsimd.dma_start(
                    t[:], agin[2048:3072, :]
                    .rearrange("(g kv) d -> g (kv d)", kv=8))
            else:
                nc.gpsimd.dma_start(
                    t[:], agout[3072 * c + 2048:3072 * c + 3072, :]
                    .rearrange("(g kv) d -> g (kv d)", kv=8))
            vvt.append(t)
        iknF = []
        for ft in range(8):
            t = mt([128, G], F32, "iknf", f"iknF{ft}", bufs=8)
            tr = t[:].rearrange("p (c g) -> p c g", c=4)
            if single_core:
                for c in range(4):
                    nc.gpsimd.dma_start(
                        tr[:, c, :],
                        agin[1024 + 128 * ft:1024 + 128 * (ft + 1), :])
            else:
                nc.gpsimd.dma_start(
                    tr, agout[:].rearrange("(c s p) g -> s p c g",
                                           c=4, s=24, p=128)[8 + ft])
            iknF.append(t)
        # ================= q path (fp16, overlaps AG/retrieval) ==========
        csqb = cload("csqb_t", [64, 2 * TC], d["csqb"][:], F16)
        qr_t = [None] * 16
        qs_t = [None] * 16

        def qb_cons(h, ps):
            # drain the PSUM bank with two parallel copies (Act + DVE) so
            # the next pass's banks free ~2x faster; rope math is deferred
            # to qb_rope after the pass. qs rides the famb ring (qa_sb gone).
            qr = mt([128, TC], F16, f"qr{h}", f"qr{h}")
            nc.scalar.copy(qr[0:64, :], ps[0:64, :])
            qs = mt([64, TC], F16, "famb", f"qstg{h}", bufs=16)
            nc.vector.tensor_copy(qs[:], ps[64:128, :])
            qr_t[h] = qr
            qs_t[h] = qs

        def qb_rope(h):
            qr, qs = qr_t[h], qs_t[h]
            t1 = mt([32, TC], F16, "qrt", f"qt1_{h}", bufs=4)
            t2 = mt([32, TC], F16, "qrt", f"qt2_{h}", bufs=4)
            nc.vector.tensor_mul(t1[:], qs[0:32, :], csqb[0:32, 0:TC])
            nc.vector.tensor_mul(t2[:], qs[32:64, :], csqb[32:64, 0:TC])
            nc.vector.tensor_add(qr[64:96, :], t1[:], t2[:])
            t3 = mt([32, TC], F16, "qrt", f"qt3_{h}", bufs=4)
            t4 = mt([32, TC], F16, "qrt", f"qt4_{h}", bufs=4)
            nc.vector.tensor_mul(t3[:], qs[0:32, :], csqb[0:32, TC:2 * TC])
            nc.vector.tensor_mul(t4[:], qs[32:64, :], csqb[32:64, TC:2 * TC])
            nc.vector.tensor_sub(qr[96:128, :], t4[:], t3[:])

        # ================= iq path (fp16-split -> fp32) =================
        iq_sb = [None] * 8

        def iq_cons(mi, ps, ps2):
            t = mt([128, TC], F32, "famc", f"iqsb{mi}", bufs=8)
            t2s = mt([128, TC], F32, "plo", f"iqlo{mi}", bufs=2)
            nc.scalar.activation(t2s[:], ps2[:], AF.Copy, scale=1.0 / LSHIFT)
            nc.vector.tensor_add(t[:], t2s[:], ps[:])
            iq_sb[mi] = t

        # interleave qb halves with iq groups: iq's PE work hides qb's
        # DVE-side rope drain.
        pass8("qb", d["wq"], 0, C, xh_s, qb_cons)
        for h in range(8):
            qb_rope(h)
        pass_idx("iq", d["iqf_w"], 0, iq_cons)
        pass8("qb", d["wq"], 1, C, xh_s, qb_cons)
        for h in range(8, 16):
            qb_rope(h)
        pass_idx("iq", d["iqf_w"], 1, iq_cons)

        ps_qssq = pt("b4", "qssq", (16, TC))
        for ft in range(8):
            iqsq = mt([128, TC], F32, "sqs", f"iqsq{ft}", bufs=2)
            nc.scalar.activation(iqsq[:], iq_sb[ft][:], AF.Square)
            nc.tensor.matmul(ps_qssq[:], ebT2[:, ft * 16:(ft + 1) * 16],
                             iqsq[:], start=(ft == 0), stop=(ft == 7))
        sq_sqrt = mt([16, TC], F32, "s_iq_a", "s_iq_a")
        nc.scalar.activation(sq_sqrt[:], ps_qssq[:], AF.Sqrt,
                             scale=1.0 / IDX_HD, bias=epsb[:16, :])
        s_iq = mt([16, TC], F32, "s_iq", "s_iq")
        nc.vector.reciprocal(s_iq[:], sq_sqrt[:])
        for ft in range(8):
            psb = pt("b6", f"qbc{ft}")
            nc.tensor.matmul(psb[:], eblk[:, ft * 128:(ft + 1) * 128], s_iq[:],
                             start=True, stop=True)
            nc.vector.tensor_mul(iq_sb[ft][:], iq_sb[ft][:], psb[:])
        # ckrF retrieval reuses the xl ring, so it must be emitted after the
        # last xl reader (the iq pass above); overlaps the topk section.
        ckrF = []
        for kv in range(8):
            t = mt([128, G], F16, "xl", f"ckrF{kv}", bufs=16)
            tr = t[:].rearrange("p (c g) -> p c g", c=4)
            if single_core:
                for c in range(4):
                    nc.gpsimd.dma_start(
                        tr[:, c, :], agin[128 * kv:128 * (kv + 1), :])
            else:
                nc.gpsimd.dma_start(
                    tr, agout[:].rearrange("(c s p) g -> s p c g",
                                           c=4, s=24, p=128)[kv])
            ckrF.append(t)


        # ---------- indexer scores + top-64 selection (fp32) ----------
        caus01T = []
        for i in range(4):
            t = mt([128, TC], F16, f"caus01T{i}", f"caus01T{i}")
            nc.scalar.dma_start(t[:], d["caus01T"][i * 128:(i + 1) * 128, :])
            caus01T.append(t)
        Mt = [mt([128, TC], F16, "xl", f"msk{gt}", bufs=16) for gt in range(4)]
        for tt in range(4):
            causadd = mt([128, G], F16, "cauadd", f"causadd{tt}", bufs=2)
            nc.scalar.dma_start(causadd[:],
                                d["causadd"][tt * 128:(tt + 1) * 128, :])
            psi = pt("b4", f"iscp{tt}", (128, G))
            for ft in range(8):
                nc.tensor.matmul(psi[:],
                                 iq_sb[ft][:, tt * 128:(tt + 1) * 128],
                                 iknF[ft][:], start=(ft == 0), stop=(ft == 7))
            isc = mt([128, G], F32, "isc", f"isc{tt}", bufs=2)
            nc.vector.scalar_tensor_tensor(isc[:], psi[:], IDX_SCALE,
                                           causadd[:], op0=ALU.mult,
                                           op1=ALU.add)
            for r in range(8):
                mx = mt([128, 8], F32, "mx", f"mx{tt}_{r}", bufs=2)
                nc.vector.max(mx[:], isc[:])
                nc.vector.match_replace(isc[:], mx[:], isc[:], ZAP)
            nc.vector.tensor_scalar(isc[:], isc[:], SEL_THR, None,
                                    op0=ALU.is_le)
            for gt in range(4):
                pst = pt("b6", f"trp{gt}_{tt}", (128, 128))
                nc.tensor.transpose(pst[:],
                                    isc[:, gt * 128:(gt + 1) * 128], ident[:])
                nc.vector.tensor_mul(Mt[gt][:, tt * 128:(tt + 1) * 128], pst[:],
                                     caus01T[gt][:, tt * 128:(tt + 1) * 128])

        # ---------- attention per head (fp16 values, fp32 softmax den) -----
        outT = []
        for h in range(NH):
            kv = h // 2
            ps_den = pt("b4", f"aden{h}", (1, TC))
            ps_out = pt(f"b{5 + 2 * (h % 2)}", f"aout{h}")
            # all 4 score matmuls first so the PE pipeline hides the
            # exp+mask latency of each pu behind the later scores
            pss, pus = [], []
            for gt in range(4):
                ps_s = pt(f"b{gt}", f"asc{h}_{gt}")
                nc.tensor.matmul(ps_s[:],
                                 ckrF[kv][:, gt * 128:(gt + 1) * 128],
                                 qr_t[h][:], start=True, stop=True)
                pss.append(ps_s)
            for gt in range(4):
                pu = mt([128, TC], F16, "pu", f"pu{h}_{gt}", bufs=6)
                nc.scalar.activation(pu[:], pss[gt][:], AF.Exp,
                                     scale=ATT_SCALE)
                nc.vector.tensor_mul(pu[:], pu[:], Mt[gt][:])
                pus.append(pu)
            for gt in range(4):
                nc.tensor.matmul(ps_den[:], oneskh[:], pus[gt][:],
                                 start=(gt == 0), stop=(gt == 3))
                nc.tensor.matmul(ps_out[:],
                                 vvt[gt][:, kv * 128:(kv + 1) * 128],
                                 pus[gt][:], start=(gt == 0), stop=(gt == 3))
            den = mt([1, TC], F32, "den", f"den{h}", bufs=2)
            nc.vector.tensor_scalar(den[:], ps_den[:], expsink[0:1, h:h + 1],
                                    None, op0=ALU.add)
            rec = mt([1, TC], F32, "rec", f"rec{h}", bufs=2)
            nc.vector.reciprocal(rec[:], den[:])
            recB = mt([128, TC], F32, "recb", f"recb{h}", bufs=1)
            nc.gpsimd.partition_broadcast(recB[:], rec[:])
            # reuse head h's qr slot: qr[h] is dead after this head's score
            # matmuls, strictly before ot is written.
            ot = mt([128, TC], F16, f"qr{h}", f"outT{h}")
            nc.vector.tensor_mul(ot[:], ps_out[:], recB[:])
            outT.append(ot)
        outT_s = [t[:] for t in outT]

        # ---------- output projection ----------
        h_sb = [None] * 8

        def owa_cons(mi, ps):
            t = mt([128, TC], F16, "famb", f"hsb{mi}", bufs=16)
            nc.vector.tensor_copy(t[:], ps[:])
            h_sb[mi] = t

        pass8("owa", d["owaT"], 0, C, outT_s, owa_cons)
        h_s = [h_sb[i][:] for i in range(8)]
        shB = mt([128, TC], F32, "shB", "shB")

        def emit_hrms():
            ps_hssq = pt("b5", "hssq", (1, TC))
            for mi in range(8):
                hsq = mt([128, TC], F16, "sqs", f"hsq{mi}", bufs=2)
                nc.scalar.activation(hsq[:], h_sb[mi][:], AF.Square)
                nc.tensor.matmul(ps_hssq[:], oneskh[:], hsq[:],
                                 start=(mi == 0), stop=(mi == 7))
            sh_sqrt = mt([1, TC], F32, "s_h_a", "s_h_a")
            nc.scalar.activation(sh_sqrt[:], ps_hssq[:], AF.Sqrt,
                                 scale=1.0 / ORPG, bias=epsb[:1, :])
            s_h = mt([1, TC], F32, "s_h", "s_h")
            nc.vector.reciprocal(s_h[:], sh_sqrt[:])
            nc.gpsimd.partition_broadcast(shB[:], s_h[:])

        # y = (h @ opb) * rms_scale  (scale factored out of the contraction)
        def opb_cons(mi, ps):
            t = mt([128, TC], F32, "yo", f"yo{mi}", bufs=2)
            nc.vector.tensor_mul(t[:], ps[:], shB[:])
            nc.scalar.dma_start(yT[mi * 128:(mi + 1) * 128, :], t[:])

        emit_hrms()
        pass8("opb", d["opb"], 0, ORPG, h_s, opb_cons)
        pass8("opb", d["opb"], 1, ORPG, h_s, opb_cons)


# ------------------------------------------------------------------
# host side
# ------------------------------------------------------------------

def make_host_constants():
    ge = np.arange(RATIO - 1, T, RATIO)             # group ends [G]
    pos = np.arange(T, dtype=np.float32)
    inv = 10000.0 ** (-np.arange(0, HD // 2, dtype=np.float32) / (HD // 2))
    ang = pos[:, None] * inv[None, :]               # [T, 64]
    cos_full = np.cos(ang).astype(np.float32)
    sin_full = np.sin(ang).astype(np.float32)
    consts = {}
    consts["eblk"] = np.zeros((16, 1024), np.float32)
    for hh in range(16):
        consts["eblk"][hh, hh * 64:(hh + 1) * 64] = 1.0
    consts["eblkT2"] = np.ascontiguousarray(
        consts["eblk"].T.reshape(8, 128, 16).transpose(1, 0, 2)
        .reshape(128, 128))
    consts["onesk"] = np.ones((128, 1), np.float32)
    consts["oneskh"] = np.ones((128, 1), F16NP)
    consts["ident"] = np.eye(128, dtype=np.float32)
    consts["identh"] = np.eye(128, dtype=F16NP)
    percore = []
    tarr = np.arange(T)
    for c in range(NCORE):
        q = c % 4
        t0 = TC * q
        g0 = GC * q
        pc = {}
        cq = cos_full[t0:t0 + TC, :32].T
        sq = sin_full[t0:t0 + TC, :32].T
        cg = cos_full[ge[g0:g0 + GC], :32].T
        sg = sin_full[ge[g0:g0 + GC], :32].T
        # [cos|sin ; sin|cos]: rows 0:32 pair with ps[64:96], 32:64 with
        # ps[96:128]; left half is the add-rope table, right the sub-rope
        pc["csqb"] = np.ascontiguousarray(np.block(
            [[cq, sq], [sq, cq]])).astype(F16NP)
        pc["csg1"] = np.ascontiguousarray(
            np.concatenate([cg, sg], 0)).astype(F16NP)
        pc["csg2"] = np.ascontiguousarray(
            np.concatenate([sg, cg], 0)).astype(F16NP)
        causal = (ge[None, :] <= tarr[t0:t0 + TC, None])   # [TC, G]
        pc["causadd"] = np.where(causal, 0.0, NEGM).astype(F16NP)
        pc["caus01T"] = np.ascontiguousarray(causal.T).astype(F16NP)
        percore.append(pc)
    return consts, percore


def _splitfuse16(w):
    """[C, 1024] fp32 -> [C, 2048] fp16 [Wh mg0|Wl mg0|Wh mg1|Wl mg1]."""
    w = np.asarray(w, np.float32)
    hi = w.astype(F16NP)
    lo = ((w - hi.astype(np.float32)) * np.float32(LSHIFT)).astype(F16NP)
    return np.ascontiguousarray(np.concatenate(
        [hi[:, :512], lo[:, :512], hi[:, 512:], lo[:, 512:]], axis=1))


_CACHED = {}


def get_program():
    if "nc" not in _CACHED:
        _CACHED["nc"] = build_program()
    return _CACHED["nc"]


def get_runner():
    """Cached jitted SPMD executable (mirrors bass2jax.run_bass_via_pjrt but
    builds the jax.jit once, so repeat calls skip retrace/relower)."""
    if "runner" in _CACHED:
        return _CACHED["runner"]
    import jax
    from jax.experimental.shard_map import shard_map
    from jax.sharding import Mesh, PartitionSpec
    import concourse.mybir as _mb
    from concourse.bass2jax import (_bass_exec_p, install_neuronx_cc_hook,
                                    partition_id_tensor)
    nc = get_program()
    install_neuronx_cc_hook()
    partition_name = (nc.partition_id_tensor.name
                      if nc.partition_id_tensor else None)
    in_names, out_names, out_avals, zero_shapes = [], [], [], []
    for alloc in nc.m.functions[0].allocations:
        if not isinstance(alloc, _mb.MemoryLocationSet):
            continue
        name = alloc.memorylocations[0].name
        if alloc.kind == "ExternalInput":
            if name != partition_name:
                in_names.append(name)
        elif alloc.kind == "ExternalOutput":
            shape = tuple(alloc.tensor_shape)
            dtype = _mb.dt.np(alloc.dtype)
            out_names.append(name)
            out_avals.append(jax.core.ShapedArray(shape, dtype))
            zero_shapes.append((shape, dtype))
    n_params = len(in_names)
    n_outs = len(out_avals)
    all_names = list(in_names) + list(out_names)
    if partition_name is not None:
        all_names.append(partition_name)
    donate = tuple(range(n_params, n_params + n_outs))

    def _body(*args):
        operands = list(args)
        if partition_name is not None:
            operands.append(partition_id_tensor())
        return tuple(_bass_exec_p.bind(
            *operands, out_avals=tuple(out_avals), in_names=tuple(all_names),
            out_names=tuple(out_names), lowering_input_output_aliases=(),
            sim_require_finite=True, sim_require_nnan=True, nc=nc))

    devices = jax.devices()[:NCORE]
    mesh = Mesh(np.asarray(devices), ("core",))
    in_specs = (PartitionSpec("core"),) * (n_params + n_outs)
    out_specs = (PartitionSpec("core"),) * n_outs
    sharded = jax.jit(
        shard_map(_body, mesh=mesh, in_specs=in_specs, out_specs=out_specs,
                  check_rep=False),
        donate_argnums=donate, keep_unused=True)

    def run(in_maps):
        concat_in = [
            np.concatenate([np.asarray(in_maps[c][nm]) for c in range(NCORE)],
                           axis=0)
            for nm in in_names]
        zeros = [np.zeros((NCORE * s[0], *s[1:]), dt)
                 for (s, dt) in zero_shapes]
        outs = sharded(*concat_in, *zeros)
        return [
            {nm: np.asarray(outs[i]).reshape(NCORE, *zero_shapes[i][0])[c]
             for i, nm in enumerate(out_names)}
            for c in range(NCORE)]

    _CACHED["runner"] = run
    return run


def kernel(x, cos, sin, q_a_w, q_b_w, ck_w, cv_w, cg_w, c_ape,
           iq_w, ik_w, ig_w, i_ape, sink, o_wa, o_pb):
    nc = get_program()
    x = np.asarray(x, np.float32)
    if "consts" not in _CACHED:
        _CACHED["consts"] = make_host_constants()
    consts, percore = _CACHED["consts"]
    c_ape = np.asarray(c_ape, np.float32)
    i_ape = np.asarray(i_ape, np.float32)
    # apegf: [128 (d within kv-head), 8 kv * RATIO]
    apegf = np.ascontiguousarray(
        c_ape.transpose(1, 2, 0).reshape(NKV, HD, RATIO)
        .transpose(1, 0, 2).reshape(HD, NKV * RATIO)).astype(np.float32)
    iape_t = i_ape.transpose(1, 2, 0).reshape(IDX_NH * IDX_HD, RATIO) \
        .reshape(8, 128, RATIO)
    iapegf = np.ascontiguousarray(
        iape_t.transpose(1, 0, 2).reshape(128, 8 * RATIO)).astype(np.float32)
    ck16 = np.asarray(ck_w, np.float32).astype(F16NP)
    cv16 = np.asarray(cv_w, np.float32).astype(F16NP)
    ckv = np.ascontiguousarray(np.concatenate(
        [ck16[:, :512], cv16[:, :512], ck16[:, 512:], cv16[:, 512:]], axis=1))
    wq = (np.asarray(q_a_w, np.float32) @
          np.asarray(q_b_w, np.float32)).astype(F16NP)
    shared = dict(
        wq=wq,
        ckv_w=ckv,
        cg_w=np.asarray(cg_w, np.float32).astype(F16NP),
        ikf_w=_splitfuse16(ik_w),
        igf_w=_splitfuse16(ig_w),
        iqf_w=_splitfuse16(iq_w),
        owaT=np.ascontiguousarray(
            np.asarray(o_wa, np.float32)[0].T).astype(F16NP),
        opb=np.asarray(o_pb, np.float32).astype(F16NP),
        apegf=apegf, iapegf=iapegf,
        sink=np.asarray(sink, np.float32).reshape(1, 16),
        **consts,
    )
    in_maps = []
    for c in range(NCORE):
        b, q = c // 4, c % 4
        m = dict(shared)
        xTc = np.ascontiguousarray(x[b, TC * q:TC * (q + 1), :].T)
        xh = xTc.astype(F16NP)
        m["xh"] = xh
        m["xl"] = ((xTc - xh.astype(np.float32)) *
                   np.float32(LSHIFT)).astype(F16NP)
        m.update(percore[c])
        in_maps.append(m)
    results = get_runner()(in_maps)
    y = np.empty((B, T, C), np.float32)
    for c in range(NCORE):
        b, q = c // 4, c % 4
        y[b, TC * q:TC * (q + 1), :] = results[c]["yT"].T
    return y


# revision 58
# speedup vs baseline: 1.1642x; 1.0002x over previous
"""DeepSeek hybrid sparse attention (CSA layer) Bass/Tile kernel for TRN2.

Sharding: 8 cores = batch (2) x sequence-chunk (4). Each core handles 512
tokens of one batch element: all projections, its slice of compressed K/V,
indexer keys; AllGather of compressed tensors within each 4-core batch
group; then dense-masked attention over the 512 compressed groups with
on-device top-64 selection; grouped output projection.

All activations on-chip are feature-major ([feature, token]) so matmuls
chain without transposes (weights stationary as lhsT).

Precision: the indexer chain (iq/ik/ig projections) runs as a 3-term fp16
hi/lo split:
    P1 = Wh.xh            (one PSUM bank)
    P2 = Wh.xl2k + Wl2k.xh  (second bank, lo parts pre-scaled by 2^11)
    W.x ~= P1 + 2^-11 P2
which carries ~22 mantissa bits (max iscore deviation vs fp32 < 1e-6,
verified to reproduce the fp32 top-64 selection exactly on this input) at
3 PE cycles/row instead of fp32's 4. Pooling, rms, iscore and top-k stay
fp32. The value chain (q/k/v, attention, output projection) runs in fp16
with fp32 accumulation; softmax denominators and rms scales in fp32.

DMA discipline: each HWDGE descriptor-generation costs ~625 ns on a shared
device, so weight strips are fused into [128, 1024] panels (ck|cv and
Wh|Wl pairs are interleaved host-side), x-hi loads as one DMA, small
constants are packed, and the post-AllGather retrievals ride the software
DGE (Pool) queue. Weight strips keep the SP queue to themselves; x,
constants, AllGather staging and output writes use the Activation queue.
"""

import numpy as np
import ml_dtypes
import concourse.bass as bass
import concourse.mybir as mybir
import concourse.tile as tile
from concourse import bacc

F32 = mybir.dt.float32
F16 = mybir.dt.float16
AF = mybir.ActivationFunctionType
ALU = mybir.AluOpType
F16NP = np.float16

# model dims
B, T, C = 2, 2048, 2048
NH, NKV, HD = 16, 8, 128
RATIO = 4
G = T // RATIO            # 512 compressed groups (full)
IDX_NH, IDX_HD = 16, 64
TOPK = 64
QR = 1024                 # q lowrank
ORPG = 1024               # o_proj rank
TC = 512                  # tokens per core
GC = 128                  # groups per core
NCORE = 8
NEGM = -30000.0           # additive causal mask value (exp -> 0 in fp32)
ZAP = -1.0e9              # top-k zap sentinel
SEL_THR = -5.0e8          # detection threshold for zapped entries
EPS = 1e-6
LSHIFT = float(2.0 ** 11)   # fp16 split lo-part scale

IDX_SCALE = float(np.float32(IDX_HD ** -0.5) / np.float32(IDX_NH))
ATT_SCALE = float(np.float32(HD ** -0.5))


def build_program(single_core=False):
    nc = bacc.Bacc("TRN2", target_bir_lowering=False, debug=False,
                   num_devices=1 if single_core else NCORE)
    dram = {}

    def din(name, shape, dtype=F32):
        dram[name] = nc.dram_tensor(name, shape, dtype, kind="ExternalInput").ap()
        return dram[name]

    din("xh", [C, TC], F16)              # fp16 hi part of x
    din("xl", [C, TC], F16)              # fp16 lo part (x - xh) * 2^11
    din("wq", [C, NH * HD], F16)         # host-fused q_a_w @ q_b_w
    din("ckv_w", [C, 2 * NKV * HD], F16)   # [ck mg0|cv mg0|ck mg1|cv mg1]
    din("cg_w", [C, NKV * HD], F16)
    din("ikf_w", [C, 2048], F16)           # [Wh mg0|Wl mg0|Wh mg1|Wl mg1]
    din("igf_w", [C, 2048], F16)
    din("iqf_w", [C, 2048], F16)
    din("owaT", [C, ORPG], F16)
    din("opb", [ORPG, C], F16)
    din("csqb", [64, 2 * TC], F16)       # [cos|sin ; sin|cos] rope table
    din("csg1", [64, GC], F16)
    din("csg2", [64, GC], F16)
    din("apegf", [128, 32], F32)         # gate ape [d, kv*R]
    din("iapegf", [128, 32], F32)        # indexer gate ape [d, ft*R]
    din("causadd", [TC, G], F16)         # token-major additive (-30000/0)
    din("caus01T", [G, TC], F16)         # g-major multiplicative (1/0)
    din("eblk", [16, 1024])              # head-block indicator
    din("eblkT2", [128, 128])            # fused ebT blocks [128, 8*16]
    din("onesk", [128, 1])
    din("oneskh", [128, 1], F16)
    din("ident", [128, 128])
    din("identh", [128, 128], F16)
    din("sink", [1, 16])
    yT = nc.dram_tensor("yT", [C, TC], F32, kind="ExternalOutput").ap()

    with tile.TileContext(nc) as tc:
        _emit(nc, tc, dram, yT, single_core=single_core)
    nc.compile()
    return nc


def _emit(nc, tc, d, yT, single_core=False):
    import contextlib
    ctx = contextlib.ExitStack()
    with ctx:
        mem = ctx.enter_context(tc.tile_pool(name="mem", bufs=1))
        psum = ctx.enter_context(tc.tile_pool(name="ps", bufs=1, space="PSUM"))
        dpool = ctx.enter_context(tc.tile_pool(name="dram", bufs=1, space="DRAM"))

        def mt(shape, dtype, tag, name, bufs=None):
            return mem.tile(shape, dtype, tag=tag, name=name, bufs=bufs)

        def pt(tag, name, shape=(128, TC), dtype=F32):
            return psum.tile(list(shape), dtype, tag=tag, name=name)

        def cload(name, shape, src, dtype=F32, eng=None):
            t = mem.tile(shape, dtype, tag=name, name=name)
            (eng or nc.scalar).dma_start(t[:], src)
            return t

        # ---------- resident x (fp16 hi in one big tile) ----------
        xhb = mt([128, 16 * TC], F16, "xhb", "xhb")
        for i in range(16):
            nc.scalar.dma_start(xhb[:, i * TC:(i + 1) * TC],
                                d["xh"][i * 128:(i + 1) * 128, :])
        xh_s = [xhb[:, i * TC:(i + 1) * TC] for i in range(16)]
        apegf = cload("apegf_t", [128, 32], d["apegf"][:])
        csg1 = mt([128, GC], F16, "csg1_t", "csg1_t")
        nc.scalar.dma_start(csg1[64:128, :], d["csg1"][:])
        csg2 = mt([128, GC], F16, "csg2_t", "csg2_t")
        nc.scalar.dma_start(csg2[64:128, :], d["csg2"][:])
        ident = cload("ident_t", [128, 128], d["ident"][:])
        identh = cload("identh_t", [128, 128], d["identh"][:], F16)
        # x lo: 16 tiles on a ring later reused by ckrF / Mt. Needed only
        # from the ik pass on, so it rides the Pool SWDGE queue and keeps
        # the Act HWDGE queue free for the compressor's PSUM drains.
        xl_t = []
        for i in range(16):
            t = mt([128, TC], F16, "xl", f"xl{i}", bufs=16)
            nc.gpsimd.dma_start(t[:], d["xl"][i * 128:(i + 1) * 128, :])
            xl_t.append(t)
        xl_s = [t[:] for t in xl_t]
        iapegf = cload("iapegf_t", [128, 32], d["iapegf"][:], eng=nc.gpsimd)
        eblk = cload("eblk_t", [16, 1024], d["eblk"][:], eng=nc.gpsimd)
        ebT2 = cload("ebT2_t", [128, 128], d["eblkT2"][:], eng=nc.gpsimd)
        onesk = cload("onesk_t", [128, 1], d["onesk"][:], eng=nc.gpsimd)
        oneskh = cload("oneskh_t", [128, 1], d["oneskh"][:], F16,
                       eng=nc.gpsimd)
        sinkt = cload("sink_t", [1, 16], d["sink"][:], eng=nc.gpsimd)
        expsink = mt([1, 16], F32, "expsink", "expsink")
        nc.scalar.activation(expsink[:], sinkt[:], AF.Exp)
        epsb = mt([128, 1], F32, "epsb", "epsb")
        nc.vector.memset(epsb[:], EPS)

        # ---------- projection passes ----------
        def pass8(pname, w, colh, K, rhs, consumer):
            """8 output tiles from w cols [colh*1024, (colh+1)*1024), one
            fused [128,1024] strip DMA per ki."""
            pss = [pt(f"b{j}", f"{pname}ps{colh}_{j}") for j in range(8)]
            nk = K // 128
            for ki in range(nk):
                ws = mt([128, 1024], F16, "wstrip", f"{pname}w{colh}_{ki}",
                        bufs=3)
                nc.sync.dma_start(
                    ws[:], w[ki * 128:(ki + 1) * 128,
                             colh * 1024:(colh + 1) * 1024])
                rt = rhs[ki]
                for j in range(8):
                    nc.tensor.matmul(pss[j][:], ws[:, j * 128:(j + 1) * 128],
                                     rt, start=(ki == 0), stop=(ki == nk - 1))
            if consumer is None:
                return pss
            for j in range(8):
                consumer(colh * 8 + j, pss[j])
            return pss

        def pass_idx(pname, wf, mg, consumer):
            """fp16-split pass: 4 out tiles, strip = [Wh | Wl] for this mg.
            P1 (b0-3) = Wh.xh ; P2 (b4-7) = Wh.xl2k + Wl2k.xh."""
            pss = [pt(f"b{j}", f"{pname}ps{mg}_{j}") for j in range(4)]
            ps2 = [pt(f"b{4 + j}", f"{pname}pl{mg}_{j}") for j in range(4)]
            for ki in range(16):
                ws = mt([128, 1024], F16, "wstrip", f"{pname}w{mg}_{ki}",
                        bufs=3)
                nc.sync.dma_start(
                    ws[:], wf[ki * 128:(ki + 1) * 128,
                              mg * 1024:(mg + 1) * 1024])
                for j in range(4):
                    nc.tensor.matmul(pss[j][:], ws[:, j * 128:(j + 1) * 128],
                                     xh_s[ki], start=(ki == 0),
                                     stop=(ki == 15))
                for j in range(4):
                    nc.tensor.matmul(ps2[j][:], ws[:, j * 128:(j + 1) * 128],
                                     xl_s[ki], start=(ki == 0), stop=False)
                    nc.tensor.matmul(ps2[j][:],
                                     ws[:, 512 + j * 128:512 + (j + 1) * 128],
                                     xh_s[ki], start=False, stop=(ki == 15))
            for j in range(4):
                consumer(mg * 4 + j, pss[j], ps2[j])

        # ================= compressor (fp16 value path) =================
        ckr_p, cvg_p = [None] * 8, [None] * 8
        kvg = {}

        def make_ckv_consumer(mg):
            def cons(j8, ps):
                # alternate the PSUM drains between Act and DVE so banks
                # free twice as fast into the next pass
                j = j8 - mg * 8
                key, idx = ("k", mg * 4 + j) if j < 4 else ("v", mg * 4 + j - 4)
                t = mt([128, TC], F16, "famb", f"{key}sb{idx}", bufs=16)
                if j % 2 == 0:
                    nc.scalar.copy(t[:], ps[:])
                else:
                    nc.vector.tensor_copy(t[:], ps[:])
                kvg[(key, idx)] = t
            return cons

        def pool_head(kv):
            g_sb = kvg[("g", kv)]
            eg = mt([128, TC], F16, "eg", f"eg{kv}", bufs=2)
            nc.scalar.activation(eg[:], g_sb[:], AF.Exp)
            # fp16 intermediates keep the DVE 2x 16-bit path (value chain)
            esum = mt([128, GC], F16, "esum", f"esum{kv}", bufs=2)
            with nc.allow_low_precision(reason="value-path 4-elem pool"):
                nc.vector.tensor_reduce(
                    esum[:], eg[:].rearrange("p (g r) -> p g r", r=RATIO),
                    axis=mybir.AxisListType.X, op=ALU.add)
            erec = mt([128, GC], F16, "erec", f"erec{kv}", bufs=2)
            with nc.allow_low_precision(reason="value-path pool softmax"):
                nc.vector.reciprocal(erec[:], esum[:])

            def pool_one(src, tag):
                kw = mt([128, TC], F16, "kw", f"kw_{tag}{kv}", bufs=1)
                nc.vector.tensor_mul(kw[:], src[:], eg[:])
                ks = mt([128, GC], F16, "ks", f"ks_{tag}{kv}", bufs=2)
                with nc.allow_low_precision(reason="value-path 4-elem pool"):
                    nc.vector.tensor_reduce(
                        ks[:], kw[:].rearrange("p (g r) -> p g r", r=RATIO),
                        axis=mybir.AxisListType.X, op=ALU.add)
                kp = mt([128, GC], F16, f"kp_{tag}", f"kp_{tag}{kv}", bufs=2)
                nc.vector.tensor_mul(kp[:], ks[:], erec[:])
                return kp

            ck_p = pool_one(kvg[("k", kv)], "k")
            cv_p = pool_one(kvg[("v", kv)], "v")

            # rope on pooled keys (rows 64:128); output fp32 for AllGather
            ckr = mt([128, GC], F32, "ckrp", f"ckr{kv}", bufs=8)
            nc.scalar.copy(ckr[0:64, :], ck_p[0:64, :])
            t1 = mt([32, GC], F16, "grt", f"rt1g{kv}", bufs=4)
            t2 = mt([32, GC], F16, "grt", f"rt2g{kv}", bufs=4)
            nc.vector.tensor_mul(t1[:], ck_p[64:96, :], csg1[64:96, :])
            nc.vector.tensor_mul(t2[:], ck_p[96:128, :], csg1[96:128, :])
            nc.vector.tensor_add(ckr[64:96, :], t1[:], t2[:])
            t3 = mt([32, GC], F16, "grt", f"rt3g{kv}", bufs=4)
            t4 = mt([32, GC], F16, "grt", f"rt4g{kv}", bufs=4)
            nc.vector.tensor_mul(t3[:], ck_p[64:96, :], csg2[64:96, :])
            nc.vector.tensor_mul(t4[:], ck_p[96:128, :], csg2[96:128, :])
            nc.vector.tensor_sub(ckr[96:128, :], t4[:], t3[:])
            ckr_p[kv] = ckr

            # transpose pooled values to g-major (fp32 for AllGather)
            pst = pt("b6", f"tps{kv}", (128, GC), dtype=F16)
            nc.tensor.transpose(pst[:], cv_p[:], identh[:])
            cvg = mt([128, GC], F32, "cvgp", f"cvg{kv}", bufs=8)
            nc.vector.tensor_copy(cvg[:], pst[:])
            cvg_p[kv] = cvg

        def cg_cons(kv, ps):
            t = mt([128, TC], F16, "gt", f"gsb{kv}", bufs=4)
            ape = apegf[:, kv * 4:(kv + 1) * 4].unsqueeze(1).to_broadcast(
                [128, GC, RATIO])
            nc.vector.tensor_add(
                t[:].rearrange("p (g r) -> p g r", r=RATIO),
                ps[:].rearrange("p (g r) -> p g r", r=RATIO), ape)
            kvg[("g", kv)] = t
            pool_head(kv)

        pass8("ckv", d["ckv_w"], 0, C, xh_s, make_ckv_consumer(0))
        pass8("ckv", d["ckv_w"], 1, C, xh_s, make_ckv_consumer(1))
        pass8("cg", d["cg_w"], 0, C, xh_s, lambda j, ps: cg_cons(j, ps))

        # ================= indexer keys (fp16-split -> fp32) =============
        iksg = {}

        def ik_cons(key):
            def cons(mi, ps, ps2):
                t = mt([128, TC], F32, "famc", f"{key}sb{mi}", bufs=8)
                t2s = mt([128, TC], F32, "plo", f"{key}lo{mi}", bufs=2)
                nc.scalar.activation(t2s[:], ps2[:], AF.Copy,
                                     scale=1.0 / LSHIFT)
                nc.vector.tensor_add(t[:], t2s[:], ps[:])
                if key == "ig":
                    ape = iapegf[:, mi * 4:(mi + 1) * 4].unsqueeze(1) \
                        .to_broadcast([128, GC, RATIO])
                    tr = t[:].rearrange("p (g r) -> p g r", r=RATIO)
                    nc.vector.tensor_add(tr, tr, ape)
                iksg[(key, mi)] = t
            return cons

        ikp_t, iksq_t = [None] * 8, [None] * 8

        def ipool(ft):
            eg = mt([128, TC], F32, "ieg", f"ieg{ft}", bufs=1)
            nc.scalar.activation(eg[:], iksg[("ig", ft)][:], AF.Exp)
            esum = mt([128, GC], F32, "esum", f"iesum{ft}", bufs=2)
            nc.vector.tensor_reduce(esum[:],
                                    eg[:].rearrange("p (g r) -> p g r", r=RATIO),
                                    axis=mybir.AxisListType.X, op=ALU.add)
            erec = mt([128, GC], F32, "erec", f"ierec{ft}", bufs=2)
            nc.vector.reciprocal(erec[:], esum[:])
            kw = mt([128, TC], F32, "ikw", f"ikw{ft}", bufs=1)
            nc.vector.tensor_mul(kw[:], iksg[("ik", ft)][:], eg[:])
            ks = mt([128, GC], F32, "ks", f"iks{ft}", bufs=2)
            nc.vector.tensor_reduce(ks[:],
                                    kw[:].rearrange("p (g r) -> p g r", r=RATIO),
                                    axis=mybir.AxisListType.X, op=ALU.add)
            ikp = mt([128, GC], F32, "iknp", f"ikp{ft}", bufs=8)
            nc.vector.tensor_mul(ikp[:], ks[:], erec[:])
            ikp_t[ft] = ikp

        for mg in range(2):
            pass_idx("ik", d["ikf_w"], mg, ik_cons("ik"))
            pass_idx("ig", d["igf_w"], mg, ik_cons("ig"))
            for j in range(4):
                ipool(mg * 4 + j)

        # rms over each idx head (64 feats): ssq via block-diag ones matmul.
        # square and accumulate alternate so the 2-slot sqs ring never blocks
        # behind the accumulation matmuls.
        ps_ssq = pt("b4", "issq", (16, GC))
        for ft in range(8):
            iksq = mt([128, GC], F32, "sqs", f"iksq{ft}", bufs=2)
            nc.scalar.activation(iksq[:], ikp_t[ft][:], AF.Square)
            nc.tensor.matmul(ps_ssq[:], ebT2[:, ft * 16:(ft + 1) * 16],
                             iksq[:], start=(ft == 0), stop=(ft == 7))
        s_sqrt = mt([16, GC], F32, "s_ik_a", "s_ik_a")
        nc.scalar.activation(s_sqrt[:], ps_ssq[:], AF.Sqrt,
                             scale=1.0 / IDX_HD, bias=epsb[:16, :])
        s_ik = mt([16, GC], F32, "s_ik", "s_ik")
        nc.vector.reciprocal(s_ik[:], s_sqrt[:])
        for ft in range(8):
            psb = pt("b6", f"ibc{ft}", (128, GC))
            nc.tensor.matmul(psb[:], eblk[:, ft * 128:(ft + 1) * 128], s_ik[:],
                             start=True, stop=True)
            nc.vector.tensor_mul(ikp_t[ft][:], ikp_t[ft][:], psb[:])

        # ---------- AllGather of (ckr | ikn | cv_gmajor), all fp32 ----------
        agin = dpool.tile([3072, GC], F32, name="agin")
        for kv in range(8):
            nc.gpsimd.dma_start(agin[128 * kv:128 * (kv + 1), :], ckr_p[kv][:])
        for ft in range(8):
            nc.gpsimd.dma_start(agin[1024 + 128 * ft:1024 + 128 * (ft + 1), :],
                                ikp_t[ft][:])
        cvsec = agin[2048:3072, :].rearrange("(g kv) d -> g kv d", kv=8)
        for kv in range(8):
            nc.gpsimd.dma_start(cvsec[:, kv, :], cvg_p[kv][:])
        if not single_core:
            agout = dpool.tile([4 * 3072, GC], F32, name="agout")
            nc.gpsimd.collective_compute(
                "AllGather", ALU.bypass,
                replica_groups=[[0, 1, 2, 3], [4, 5, 6, 7]],
                ins=[agin.opt()], outs=[agout.opt()],
            )

        # ---------- retrieve gathered tensors (SWDGE / Pool queue) --------
        # ckrF / Mt reuse the xl ring (xl dies at the end of the iq pass).
        vvt = []
        for c in range(4):
            t = mt([128, 1024], F16, "vvt", f"vvt{c}", bufs=4)
            if single_core:
                nc.gpsimd.dma_start(
                    t[:], agin[2048:3072, :]
                    .rearrange("(g kv) d -> g (kv d)", kv=8))
            else:
                nc.gpsimd.dma_start(
                    t[:], agout[3072 * c + 2048:3072 * c + 3072, :]
                    .rearrange("(g kv) d -> g (kv d)", kv=8))
            vvt.append(t)
        iknF = []
        for ft in range(8):
            t = mt([128, G], F32, "iknf", f"iknF{ft}", bufs=8)
            tr = t[:].rearrange("p (c g) -> p c g", c=4)
            if single_core:
                for c in range(4):
                    nc.gpsimd.dma_start(
                        tr[:, c, :],
                        agin[1024 + 128 * ft:1024 + 128 * (ft + 1), :])
            else:
                nc.gpsimd.dma_start(
                    tr, agout[:].rearrange("(c s p) g -> s p c g",
                                           c=4, s=24, p=128)[8 + ft])
            iknF.append(t)
        # ================= q path (fp16, overlaps AG/retrieval) ==========
        csqb = cload("csqb_t", [64, 2 * TC], d["csqb"][:], F16)
        qr_t = [None] * 16
        qs_t = [None] * 16

        def qb_cons(h, ps):
            # drain the PSUM bank with two parallel copies (Act + DVE) so
            # the next pass's banks free ~2x faster; rope math is deferred
            # to qb_rope after the pass. qs rides the famb ring (qa_sb gone).
            qr = mt([128, TC], F16, f"qr{h}", f"qr{h}")
            nc.scalar.copy(qr[0:64, :], ps[0:64, :])
            qs = mt([64, TC], F16, "famb", f"qstg{h}", bufs=16)
            nc.vector.tensor_copy(qs[:], ps[64:128, :])
            qr_t[h] = qr
            qs_t[h] = qs

        def qb_rope(h):
            qr, qs = qr_t[h], qs_t[h]
            t1 = mt([32, TC], F16, "qrt", f"qt1_{h}", bufs=4)
            t2 = mt([32, TC], F16, "qrt", f"qt2_{h}", bufs=4)
            nc.vector.tensor_mul(t1[:], qs[0:32, :], csqb[0:32, 0:TC])
            nc.vector.tensor_mul(t2[:], qs[32:64, :], csqb[32:64, 0:TC])
            nc.vector.tensor_add(qr[64:96, :], t1[:], t2[:])
            t3 = mt([32, TC], F16, "qrt", f"qt3_{h}", bufs=4)
            t4 = mt([32, TC], F16, "qrt", f"qt4_{h}", bufs=4)
            nc.vector.tensor_mul(t3[:], qs[0:32, :], csqb[0:32, TC:2 * TC])
            nc.vector.tensor_mul(t4[:], qs[32:64, :], csqb[32:64, TC:2 * TC])
            nc.vector.tensor_sub(qr[96:128, :], t4[:], t3[:])

        # ================= iq path (fp16-split -> fp32) =================
        iq_sb = [None] * 8

        def iq_cons(mi, ps, ps2):
            t = mt([128, TC], F32, "famc", f"iqsb{mi}", bufs=8)
            t2s = mt([128, TC], F32, "plo", f"iqlo{mi}", bufs=2)
            nc.scalar.activation(t2s[:], ps2[:], AF.Copy, scale=1.0 / LSHIFT)
            nc.vector.tensor_add(t[:], t2s[:], ps[:])
            iq_sb[mi] = t

        # interleave qb halves with iq groups: iq's PE work hides qb's
        # DVE-side rope drain.
        pass8("qb", d["wq"], 0, C, xh_s, qb_cons)
        for h in range(8):
            qb_rope(h)
        pass_idx("iq", d["iqf_w"], 0, iq_cons)
        pass8("qb", d["wq"], 1, C, xh_s, qb_cons)
        for h in range(8, 16):
            qb_rope(h)
        pass_idx("iq", d["iqf_w"], 1, iq_cons)

        ps_qssq = pt("b4", "qssq", (16, TC))
        for ft in range(8):
            iqsq = mt([128, TC], F32, "sqs", f"iqsq{ft}", bufs=2)
            nc.scalar.activation(iqsq[:], iq_sb[ft][:], AF.Square)
            nc.tensor.matmul(ps_qssq[:], ebT2[:, ft * 16:(ft + 1) * 16],
                             iqsq[:], start=(ft == 0), stop=(ft == 7))
        sq_sqrt = mt([16, TC], F32, "s_iq_a", "s_iq_a")
        nc.scalar.activation(sq_sqrt[:], ps_qssq[:], AF.Sqrt,
                             scale=1.0 / IDX_HD, bias=epsb[:16, :])
        s_iq = mt([16, TC], F32, "s_iq", "s_iq")
        nc.vector.reciprocal(s_iq[:], sq_sqrt[:])
        for ft in range(8):
            psb = pt("b6", f"qbc{ft}")
            nc.tensor.matmul(psb[:], eblk[:, ft * 128:(ft + 1) * 128], s_iq[:],
                             start=True, stop=True)
            nc.vector.tensor_mul(iq_sb[ft][:], iq_sb[ft][:], psb[:])
        # ckrF retrieval reuses the xl ring, so it must be emitted after the
        # last xl reader (the iq pass above); overlaps the topk section.
        ckrF = []
        for kv in range(8):
            t = mt([128, G], F16, "xl", f"ckrF{kv}", bufs=16)
            tr = t[:].rearrange("p (c g) -> p c g", c=4)
            if single_core:
                for c in range(4):
                    nc.gpsimd.dma_start(
                        tr[:, c, :], agin[128 * kv:128 * (kv + 1), :])
            else:
                nc.gpsimd.dma_start(
                    tr, agout[:].rearrange("(c s p) g -> s p c g",
                                           c=4, s=24, p=128)[kv])
            ckrF.append(t)


        # ---------- indexer scores + top-64 selection (fp32) ----------
        caus01T = []
        for i in range(4):
            t = mt([128, TC], F16, f"caus01T{i}", f"caus01T{i}")
            nc.scalar.dma_start(t[:], d["caus01T"][i * 128:(i + 1) * 128, :])
            caus01T.append(t)
        Mt = [mt([128, TC], F16, "xl", f"msk{gt}", bufs=16) for gt in range(4)]
        for tt in range(4):
            causadd = mt([128, G], F16, "cauadd", f"causadd{tt}", bufs=2)
            nc.scalar.dma_start(causadd[:],
                                d["causadd"][tt * 128:(tt + 1) * 128, :])
            psi = pt("b4", f"iscp{tt}", (128, G))
            for ft in range(8):
                nc.tensor.matmul(psi[:],
                                 iq_sb[ft][:, tt * 128:(tt + 1) * 128],
                                 iknF[ft][:], start=(ft == 0), stop=(ft == 7))
            isc = mt([128, G], F32, "isc", f"isc{tt}", bufs=2)
            nc.vector.scalar_tensor_tensor(isc[:], psi[:], IDX_SCALE,
                                           causadd[:], op0=ALU.mult,
                                           op1=ALU.add)
            for r in range(8):
                mx = mt([128, 8], F32, "mx", f"mx{tt}_{r}", bufs=2)
                nc.vector.max(mx[:], isc[:])
                nc.vector.match_replace(isc[:], mx[:], isc[:], ZAP)
            nc.vector.tensor_scalar(isc[:], isc[:], SEL_THR, None,
                                    op0=ALU.is_le)
            for gt in range(4):
                pst = pt("b6", f"trp{gt}_{tt}", (128, 128))
                nc.tensor.transpose(pst[:],
                                    isc[:, gt * 128:(gt + 1) * 128], ident[:])
                nc.vector.tensor_mul(Mt[gt][:, tt * 128:(tt + 1) * 128], pst[:],
                                     caus01T[gt][:, tt * 128:(tt + 1) * 128])

        # ---------- attention per head (fp16 values, fp32 softmax den) -----
        outT = []
        for h in range(NH):
            kv = h // 2
            ps_den = pt("b4", f"aden{h}", (1, TC))
            ps_out = pt(f"b{5 + 2 * (h % 2)}", f"aout{h}")
            # all 4 score matmuls first so the PE pipeline hides the
            # exp+mask latency of each pu behind the later scores
            pss, pus = [], []
            for gt in range(4):
                ps_s = pt(f"b{gt}", f"asc{h}_{gt}")
                nc.tensor.matmul(ps_s[:],
                                 ckrF[kv][:, gt * 128:(gt + 1) * 128],
                                 qr_t[h][:], start=True, stop=True)
                pss.append(ps_s)
            for gt in range(4):
                pu = mt([128, TC], F16, "pu", f"pu{h}_{gt}", bufs=6)
                nc.scalar.activation(pu[:], pss[gt][:], AF.Exp,
                                     scale=ATT_SCALE)
                nc.vector.tensor_mul(pu[:], pu[:], Mt[gt][:])
                pus.append(pu)
            for gt in range(4):
                nc.tensor.matmul(ps_den[:], oneskh[:], pus[gt][:],
                                 start=(gt == 0), stop=(gt == 3))
                nc.tensor.matmul(ps_out[:],
                                 vvt[gt][:, kv * 128:(kv + 1) * 128],
                                 pus[gt][:], start=(gt == 0), stop=(gt == 3))
            den = mt([1, TC], F32, "den", f"den{h}", bufs=2)
            nc.vector.tensor_scalar(den[:], ps_den[:], expsink[0:1, h:h + 1],
                                    None, op0=ALU.add)
            rec = mt([1, TC], F32, "rec", f"rec{h}", bufs=2)
            nc.vector.reciprocal(rec[:], den[:])
            recB = mt([128, TC], F32, "recb", f"recb{h}", bufs=1)
            nc.gpsimd.partition_broadcast(recB[:], rec[:])
            # reuse head h's qr slot: qr[h] is dead after this head's score
            # matmuls, strictly before ot is written.
            ot = mt([128, TC], F16, f"qr{h}", f"outT{h}")
            nc.vector.tensor_mul(ot[:], ps_out[:], recB[:])
            outT.append(ot)
        outT_s = [t[:] for t in outT]

        # ---------- output projection ----------
        h_sb = [None] * 8

        def owa_cons(mi, ps):
            t = mt([128, TC], F16, "famb", f"hsb{mi}", bufs=16)
            nc.vector.tensor_copy(t[:], ps[:])
            h_sb[mi] = t

        pass8("owa", d["owaT"], 0, C, outT_s, owa_cons)
        h_s = [h_sb[i][:] for i in range(8)]
        shB = mt([128, TC], F32, "shB", "shB")

        def emit_hrms():
            ps_hssq = pt("b5", "hssq", (1, TC))
            for mi in range(8):
                hsq = mt([128, TC], F16, "sqs", f"hsq{mi}", bufs=2)
                nc.scalar.activation(hsq[:], h_sb[mi][:], AF.Square)
                nc.tensor.matmul(ps_hssq[:], oneskh[:], hsq[:],
                                 start=(mi == 0), stop=(mi == 7))
            sh_sqrt = mt([1, TC], F32, "s_h_a", "s_h_a")
            nc.scalar.activation(sh_sqrt[:], ps_hssq[:], AF.Sqrt,
                                 scale=1.0 / ORPG, bias=epsb[:1, :])
            s_h = mt([1, TC], F32, "s_h", "s_h")
            nc.vector.reciprocal(s_h[:], sh_sqrt[:])
            nc.gpsimd.partition_broadcast(shB[:], s_h[:])

        # y = (h @ opb) * rms_scale  (scale factored out of the contraction)
        def opb_cons(mi, ps):
            t = mt([128, TC], F32, "yo", f"yo{mi}", bufs=2)
            nc.vector.tensor_mul(t[:], ps[:], shB[:])
            nc.scalar.dma_start(yT[mi * 128:(mi + 1) * 128, :], t[:])

        emit_hrms()
        pass8("opb", d["opb"], 0, ORPG, h_s, opb_cons)
        pass8("opb", d["opb"], 1, ORPG, h_s, opb_cons)


# ------------------------------------------------------------------
# host side
# ------------------------------------------------------------------

def make_host_constants():
    ge = np.arange(RATIO - 1, T, RATIO)             # group ends [G]
    pos = np.arange(T, dtype=np.float32)
    inv = 10000.0 ** (-np.arange(0, HD // 2, dtype=np.float32) / (HD // 2))
    ang = pos[:, None] * inv[None, :]               # [T, 64]
    cos_full = np.cos(ang).astype(np.float32)
    sin_full = np.sin(ang).astype(np.float32)
    consts = {}
    consts["eblk"] = np.zeros((16, 1024), np.float32)
    for hh in range(16):
        consts["eblk"][hh, hh * 64:(hh + 1) * 64] = 1.0
    consts["eblkT2"] = np.ascontiguousarray(
        consts["eblk"].T.reshape(8, 128, 16).transpose(1, 0, 2)
        .reshape(128, 128))
    consts["onesk"] = np.ones((128, 1), np.float32)
    consts["oneskh"] = np.ones((128, 1), F16NP)
    consts["ident"] = np.eye(128, dtype=np.float32)
    consts["identh"] = np.eye(128, dtype=F16NP)
    percore = []
    tarr = np.arange(T)
    for c in range(NCORE):
        q = c % 4
        t0 = TC * q
        g0 = GC * q
        pc = {}
        cq = cos_full[t0:t0 + TC, :32].T
        sq = sin_full[t0:t0 + TC, :32].T
        cg = cos_full[ge[g0:g0 + GC], :32].T
        sg = sin_full[ge[g0:g0 + GC], :32].T
        # [cos|sin ; sin|cos]: rows 0:32 pair with ps[64:96], 32:64 with
        # ps[96:128]; left half is the add-rope table, right the sub-rope
        pc["csqb"] = np.ascontiguousarray(np.block(
            [[cq, sq], [sq, cq]])).astype(F16NP)
        pc["csg1"] = np.ascontiguousarray(
            np.concatenate([cg, sg], 0)).astype(F16NP)
        pc["csg2"] = np.ascontiguousarray(
            np.concatenate([sg, cg], 0)).astype(F16NP)
        causal = (ge[None, :] <= tarr[t0:t0 + TC, None])   # [TC, G]
        pc["causadd"] = np.where(causal, 0.0, NEGM).astype(F16NP)
        pc["caus01T"] = np.ascontiguousarray(causal.T).astype(F16NP)
        percore.append(pc)
    return consts, percore


def _splitfuse16(w):
    """[C, 1024] fp32 -> [C, 2048] fp16 [Wh mg0|Wl mg0|Wh mg1|Wl mg1]."""
    w = np.asarray(w, np.float32)
    hi = w.astype(F16NP)
    lo = ((w - hi.astype(np.float32)) * np.float32(LSHIFT)).astype(F16NP)
    return np.ascontiguousarray(np.concatenate(
        [hi[:, :512], lo[:, :512], hi[:, 512:], lo[:, 512:]], axis=1))


_CACHED = {}


def get_program():
    if "nc" not in _CACHED:
        _CACHED["nc"] = build_program()
    return _CACHED["nc"]


def get_runner():
    """Cached jitted SPMD executable (mirrors bass2jax.run_bass_via_pjrt but
    builds the jax.jit once, so repeat calls skip retrace/relower)."""
    if "runner" in _CACHED:
        return _CACHED["runner"]
    import jax
    from jax.experimental.shard_map import shard_map
    from jax.sharding import Mesh, PartitionSpec
    import concourse.mybir as _mb
    from concourse.bass2jax import (_bass_exec_p, install_neuronx_cc_hook,
                                    partition_id_tensor)
    nc = get_program()
    install_neuronx_cc_hook()
    partition_name = (nc.partition_id_tensor.name
                      if nc.partition_id_tensor else None)
    in_names, out_names, out_avals, zero_shapes = [], [], [], []
    for alloc in nc.m.functions[0].allocations:
        if not isinstance(alloc, _mb.MemoryLocationSet):
            continue
        name = alloc.memorylocations[0].name
        if alloc.kind == "ExternalInput":
            if name != partition_name:
                in_names.append(name)
        elif alloc.kind == "ExternalOutput":
            shape = tuple(alloc.tensor_shape)
            dtype = _mb.dt.np(alloc.dtype)
            out_names.append(name)
            out_avals.append(jax.core.ShapedArray(shape, dtype))
            zero_shapes.append((shape, dtype))
    n_params = len(in_names)
    n_outs = len(out_avals)
    all_names = list(in_names) + list(out_names)
    if partition_name is not None:
        all_names.append(partition_name)
    donate = tuple(range(n_params, n_params + n_outs))

    def _body(*args):
        operands = list(args)
        if partition_name is not None:
            operands.append(partition_id_tensor())
        return tuple(_bass_exec_p.bind(
            *operands, out_avals=tuple(out_avals), in_names=tuple(all_names),
            out_names=tuple(out_names), lowering_input_output_aliases=(),
            sim_require_finite=True, sim_require_nnan=True, nc=nc))

    devices = jax.devices()[:NCORE]
    mesh = Mesh(np.asarray(devices), ("core",))
    in_specs = (PartitionSpec("core"),) * (n_params + n_outs)
    out_specs = (PartitionSpec("core"),) * n_outs
    sharded = jax.jit(
        shard_map(_body, mesh=mesh, in_specs=in_specs, out_specs=out_specs,
                  check_rep=False),
        donate_argnums=donate, keep_unused=True)

    def run(in_maps):
        concat_in = [
            np.concatenate([np.asarray(in_maps[c][nm]) for c in range(NCORE)],
                           axis=0)
            for nm in in_names]
        zeros = [np.zeros((NCORE * s[0], *s[1:]), dt)
                 for (s, dt) in zero_shapes]
        outs = sharded(*concat_in, *zeros)
        return [
            {nm: np.asarray(outs[i]).reshape(NCORE, *zero_shapes[i][0])[c]
             for i, nm in enumerate(out_names)}
            for c in range(NCORE)]

    _CACHED["runner"] = run
    return run


def kernel(x, cos, sin, q_a_w, q_b_w, ck_w, cv_w, cg_w, c_ape,
           iq_w, ik_w, ig_w, i_ape, sink, o_wa, o_pb):
    nc = get_program()
    x = np.asarray(x, np.float32)
    if "consts" not in _CACHED:
        _CACHED["consts"] = make_host_constants()
    consts, percore = _CACHED["consts"]
    c_ape = np.asarray(c_ape, np.float32)
    i_ape = np.asarray(i_ape, np.float32)
    # apegf: [128 (d within kv-head), 8 kv * RATIO]
    apegf = np.ascontiguousarray(
        c_ape.transpose(1, 2, 0).reshape(NKV, HD, RATIO)
        .transpose(1, 0, 2).reshape(HD, NKV * RATIO)).astype(np.float32)
    iape_t = i_ape.transpose(1, 2, 0).reshape(IDX_NH * IDX_HD, RATIO) \
        .reshape(8, 128, RATIO)
    iapegf = np.ascontiguousarray(
        iape_t.transpose(1, 0, 2).reshape(128, 8 * RATIO)).astype(np.float32)
    ck16 = np.asarray(ck_w, np.float32).astype(F16NP)
    cv16 = np.asarray(cv_w, np.float32).astype(F16NP)
    ckv = np.ascontiguousarray(np.concatenate(
        [ck16[:, :512], cv16[:, :512], ck16[:, 512:], cv16[:, 512:]], axis=1))
    wq = (np.asarray(q_a_w, np.float32) @
          np.asarray(q_b_w, np.float32)).astype(F16NP)
    shared = dict(
        wq=wq,
        ckv_w=ckv,
        cg_w=np.asarray(cg_w, np.float32).astype(F16NP),
        ikf_w=_splitfuse16(ik_w),
        igf_w=_splitfuse16(ig_w),
        iqf_w=_splitfuse16(iq_w),
        owaT=np.ascontiguousarray(
            np.asarray(o_wa, np.float32)[0].T).astype(F16NP),
        opb=np.asarray(o_pb, np.float32).astype(F16NP),
        apegf=apegf, iapegf=iapegf,
        sink=np.asarray(sink, np.float32).reshape(1, 16),
        **consts,
    )
    in_maps = []
    for c in range(NCORE):
        b, q = c // 4, c % 4
        m = dict(shared)
        xTc = np.ascontiguousarray(x[b, TC * q:TC * (q + 1), :].T)
        xh = xTc.astype(F16NP)
        m["xh"] = xh
        m["xl"] = ((xTc - xh.astype(np.float32)) *
                   np.float32(LSHIFT)).astype(F16NP)
        m.update(percore[c])
        in_maps.append(m)
    results = get_runner()(in_maps)
    y = np.empty((B, T, C), np.float32)
    for c in range(NCORE):
        b, q = c // 4, c % 4
        y[b, TC * q:TC * (q + 1), :] = results[c]["yT"].T
    return y
